# revision 1
# baseline (speedup 1.0000x reference)
"""Trainium2 Bass kernel for nn_Attention_56736517980393.

Reference computation (B=4, S=2048, C=1024, H=16 heads, D=64, MLP hidden 1024):
    q = (x @ Wq + bq) * D**-0.5          per-head [B,H,S,D]
    k = data @ Wk + bk ; v = data @ Wv + bv
    kv[b,h] = k^T @ v                     [D,D]   (no softmax -> associative form)
    attn = q @ kv                         [B,S,C]
    out = x + attn + gelu(attn @ W1 + b1) @ W2 + b2

Sharding: 8 cores = (batch b in 0..3) x (sequence half j in 0..1).
Each core computes K/V projections for its (b, j) sequence half, a partial
kv (reduced over its half), AllReduces kv with its pair core, then computes
Q / attn / MLP / residual for its half.  Activations are kept feature-major
(transposed, [C, S]) on chip so that biases are per-partition and no on-chip
transposes are needed; K and V are produced sequence-major for the kv matmul
by using the activation tile as the stationary matmul operand.

All matmul operands are bf16 (psum accumulation is fp32), which halves DMA
traffic and SBUF footprint vs fp32r so every weight/activation tile is
prefetched long before use.  Measured end-to-end max-rel error vs the fp32
reference is ~4.6e-3 (harness budget 2e-2); GDT=f16 gives ~9e-4 at ~4%
more runtime.

DMA queues: bulk loads ride the SP queue in exact consumption order;
the kv AllReduce staging and output stores ride the GpSimd SWDGE (the
Pool sequencer is otherwise idle, and Act-queue DMA issues cost ~1.6us
each, which was stalling next-iteration K/V evictions behind the stores).
Phase-1 PSUM evictions split across Act and DVE.
"""

import os
import numpy as np

GDT = os.environ.get("GDT", "bf16")

B, S, C, H, D = 4, 2048, 1024, 16, 64
SH = S // 2          # sequence rows per core
SCALE = D ** -0.5
P = 128              # SBUF partitions
NB = 512             # matmul moving free-dim block (one PSUM bank of fp32)
KT = C // P          # 8 contraction tiles
MT = C // P          # 8 output-feature tiles
NBLK = SH // NB      # 2 sequence blocks

_CACHE = {}


def _build(with_bkv: bool, loop_r: int = 1, n_cores: int = 8, use_cc: bool = True, phases: int = 4, mmx: int = 1, gelu: bool = True):
    import concourse.bacc as bacc
    import concourse.mybir as mybir
    from concourse.tile import TileContext

    F32 = mybir.dt.float32
    F16 = {"f16": mybir.dt.float16, "bf16": mybir.dt.bfloat16,
           "f32r": mybir.dt.float32r}[GDT]
    ODT = mybir.dt.float16
    AF = mybir.ActivationFunctionType
    ALU = mybir.AluOpType

    nc = bacc.Bacc(
        trn_type="TRN2", target_bir_lowering=False, debug=False, num_devices=n_cores
    )

    xT = nc.dram_tensor("xT", [C, SH], F16, kind="ExternalInput").ap()
    dT = nc.dram_tensor("dT", [C, SH], F16, kind="ExternalInput").ap()
    wq = nc.dram_tensor("wq", [C, C], F16, kind="ExternalInput").ap()
    wk = nc.dram_tensor("wk", [C, C], F16, kind="ExternalInput").ap()
    wv = nc.dram_tensor("wv", [C, C], F16, kind="ExternalInput").ap()
    w1 = nc.dram_tensor("w1", [C, C], F16, kind="ExternalInput").ap()
    w2 = nc.dram_tensor("w2", [C, C], F16, kind="ExternalInput").ap()
    # feature-major biases: [128, 8] so that column m is the per-partition
    # bias for feature tile m
    bqt = nc.dram_tensor("bqt", [P, MT], F32, kind="ExternalInput").ap()
    b1t = nc.dram_tensor("b1t", [P, MT], F32, kind="ExternalInput").ap()
    b2t = nc.dram_tensor("b2t", [P, MT], F32, kind="ExternalInput").ap()
    if with_bkv:
        bkr = nc.dram_tensor("bkr", [1, C], F16, kind="ExternalInput").ap()
        bvr = nc.dram_tensor("bvr", [1, C], F16, kind="ExternalInput").ap()
    outT = nc.dram_tensor("outT", [C, SH], ODT, kind="ExternalOutput").ap()

    groups = [[i, i + 1] for i in range(0, n_cores, 2)]

    with TileContext(nc) as tc:
        with tc.tile_pool(name="sb", bufs=1) as sb, \
             tc.tile_pool(name="ps", bufs=1, space="PSUM") as psp, \
             tc.tile_pool(name="dr", bufs=1, space="DRAM") as drp:

            # One-time act-table preload: the gelu_and_others set also holds
            # Identity and Copy, so every later Act op is served without a
            # 1.3us table swap.
            if gelu:
                warm = sb.tile([1, 8], F32, tag="warm", bufs=1, name="warm")
                nc.vector.memset(warm[:], 0.0)
                nc.scalar.activation(warm[:], warm[:], AF.Gelu, scale=1.0)

            def body(it=None):
                # ---- SBUF tiles ----
                wk_sb = [sb.tile([P, C], F16, tag="wt", bufs=40, name=f"wk{i}")
                         for i in range(KT)]
                wv_sb = [sb.tile([P, C], F16, tag="wt", bufs=40, name=f"wv{i}")
                        for i in range(KT)]
                dT_sb = [sb.tile([P, SH], F16, tag="dh", bufs=8, name=f"dT{i}")
                         for i in range(KT)]
                xT_sb = [sb.tile([P, SH], F16, tag="xa", bufs=8, name=f"xT{i}")
                         for i in range(KT)]
                # ---- SP load queue: strict phase-1 consumption order, then
                # next-phase prefetch.  Everything is resident well before use.
                for i in range(KT):
                    nc.sync.dma_start(out=dT_sb[i][:, 0:P], in_=dT[i * P:(i + 1) * P, 0:P])
                    nc.sync.dma_start(out=wk_sb[i][:], in_=wk[i * P:(i + 1) * P, :])
                for i in range(KT):
                    nc.sync.dma_start(out=dT_sb[i][:, P:SH], in_=dT[i * P:(i + 1) * P, P:SH])
                for i in range(KT):
                    nc.sync.dma_start(out=wv_sb[i][:], in_=wv[i * P:(i + 1) * P, :])
                bq_sb = sb.tile([P, MT], F32, tag="bias", bufs=3, name="bq_sb")
                b1_sb = sb.tile([P, MT], F32, tag="bias", bufs=3, name="b1_sb")
                b2_sb = sb.tile([P, MT], F32, tag="bias", bufs=3, name="b2_sb")
                nc.sync.dma_start(out=bq_sb[:], in_=bqt[:])
                nc.sync.dma_start(out=b1_sb[:], in_=b1t[:])
                nc.sync.dma_start(out=b2_sb[:], in_=b2t[:])
                if with_bkv:
                    bk_sb = sb.tile([1, C], F16, tag="brow", bufs=2, name="bk_sb")
                    bv_sb = sb.tile([1, C], F16, tag="brow", bufs=2, name="bv_sb")
                    ones = sb.tile([1, P], F16, tag="ones", bufs=1, name="ones")
                    nc.sync.dma_start(out=bk_sb[:], in_=bkr[:])
                    nc.sync.dma_start(out=bv_sb[:], in_=bvr[:])
                    nc.vector.memset(ones[:], 1.0)
                wq_sb = [sb.tile([P, C], F16, tag="wt", bufs=40, name=f"wq{i}")
                         for i in range(KT)]
                w1_sb = [sb.tile([P, C], F16, tag="wt", bufs=40, name=f"w1{i}")
                         for i in range(KT)]
                w2_sb = [sb.tile([P, C], F16, tag="wt", bufs=40, name=f"w2{i}")
                         for i in range(KT)]
                for i in range(KT):
                    nc.sync.dma_start(out=xT_sb[i][:], in_=xT[i * P:(i + 1) * P, :])
                    nc.sync.dma_start(out=wq_sb[i][:], in_=wq[i * P:(i + 1) * P, :])
                for i in range(KT):
                    nc.sync.dma_start(out=w1_sb[i][:], in_=w1[i * P:(i + 1) * P, :])
                for i in range(KT):
                    nc.sync.dma_start(out=w2_sb[i][:], in_=w2[i * P:(i + 1) * P, :])

                # ---- phase 1: K/V (sequence-major) and kv partial ----
                # Every dT stationary serves both feature halves (paired
                # matmuls into two PSUM banks) so the PE loads each
                # stationary once per two matmuls.  kv likewise pairs two
                # heads per [128,128] k stationary; the unwanted cross-head
                # half of each product lands in the unused partition half of
                # its bank (bank A keeps even heads at partitions 0:64,
                # bank B odd heads at 64:128, head pair i at free i*64).
                kv_ps_a = psp.tile([P, NB], F32, tag="acc", bufs=2, name="kv_ps_a")
                kv_ps_b = psp.tile([P, NB], F32, tag="acc", bufs=2, name="kv_ps_b")

                def proj_seq(w_sb, brow, st, nm):
                    ssl = slice(st * P, (st + 1) * P)
                    t = sb.tile([P, C], F16, tag="kvs", bufs=5, name=nm)
                    pk0 = psp.tile([P, NB], F32, tag="mm", bufs=6, name="pk0")
                    pk1 = psp.tile([P, NB], F32, tag="mm", bufs=6, name="pk1")
                    for kt in range(KT):
                        for nb, pk in ((0, pk0), (1, pk1)):
                            nc.tensor.matmul(
                                pk[:],
                                lhsT=dT_sb[kt][:, ssl],
                                rhs=w_sb[kt][:, nb * NB:(nb + 1) * NB],
                                start=(kt == 0),
                                stop=(kt == KT - 1 and not with_bkv),
                            )
                    if with_bkv:
                        nc.tensor.matmul(pk0[:], lhsT=ones[:],
                                         rhs=brow[:, 0:NB],
                                         start=False, stop=True)
                        nc.tensor.matmul(pk1[:], lhsT=ones[:],
                                         rhs=brow[:, NB:C],
                                         start=False, stop=True)
                    nc.scalar.copy(t[:, 0:NB], pk0[:])
                    nc.vector.tensor_copy(t[:, NB:C], pk1[:])
                    return t

                def kv_mms(k_t, v_t, st):
                    # pair i: full [128,128] product of the pair's k and v
                    # slices.  Quadrants [0:64,0:64] and [64:128,64:128] are
                    # the two heads' kv; the off-diagonal quadrants are
                    # cross-head junk, zeroed after the AllReduce.
                    for i in range(8):
                        tgt = kv_ps_a if i < 4 else kv_ps_b
                        fo = (i % 4) * 128
                        nc.tensor.matmul(
                            tgt[:, fo:fo + 128],
                            lhsT=k_t[:, i * 128:(i + 1) * 128],
                            rhs=v_t[:, i * 128:(i + 1) * 128],
                            start=(st == 0 and i % 4 == 0),
                            stop=(st == MT - 1 and i % 4 == 3),
                        )

                pend = None
                for st in range(MT):
                    k_t = proj_seq(wk_sb, bk_sb if with_bkv else None, st, "k_t")
                    if pend is not None:
                        kv_mms(*pend)
                    v_t = proj_seq(wv_sb, bv_sb if with_bkv else None, st, "v_t")
                    pend = (k_t, v_t, st)
                kv_mms(*pend)

                # ---- phase 2: AllReduce kv across the pair ----
                # kv_sb [128, 1024]: pair i at cols i*128 (junk rides along;
                # junk+junk is still junk and gets zeroed after the reduce).
                kv_sb = sb.tile([P, C], F16, tag="kvio", bufs=2, name="kv_sb")
                nc.vector.tensor_copy(kv_sb[:, 0:NB], kv_ps_a[:])
                nc.vector.tensor_copy(kv_sb[:, NB:C], kv_ps_b[:])
                kv_in = drp.tile([P, C], F16, tag="dri", bufs=1, name="kv_in")
                kv_out = drp.tile([P, C], F16, tag="dro", bufs=1, name="kv_out")
                nc.gpsimd.dma_start(out=kv_in[:], in_=kv_sb[:])
                if n_cores == 1 or not use_cc:
                    # single-core analysis build: stand in for the AllReduce
                    nc.gpsimd.dma_start(out=kv_out[:], in_=kv_in[:])
                else:
                    nc.gpsimd.collective_compute(
                        "AllReduce", ALU.add, replica_groups=groups,
                        ins=[kv_in.opt()], outs=[kv_out.opt()],
                    )
                kv_red = sb.tile([P, C], F16, tag="kvr", bufs=2, name="kv_red")
                nc.gpsimd.dma_start(out=kv_red[:], in_=kv_out[:])
                # zero the cross-head quadrants so attention can contract
                # over the full 128 partitions in one matmul per pair
                for i in range(8):
                    co = i * 128
                    nc.vector.memset(kv_red[64:128, co:co + 64], 0.0)
                    nc.vector.memset(kv_red[0:64, co + 64:co + 128], 0.0)

                if phases <= 1:
                    stg0 = sb.tile([P, NB], ODT, tag="stg", bufs=3, name="stg0")
                    nc.vector.tensor_copy(stg0[:], kv_red[:, 0:NB])
                    for m in range(MT):
                        for nb in range(NBLK):
                            nsl = slice(nb * NB, (nb + 1) * NB)
                            nc.gpsimd.dma_start(out=outT[m * P:(m + 1) * P, nsl],
                                                in_=stg0[:])
                    return

                # ---- phase 3: Q projection (all blocks), then attention ----
                # All 16 Q blocks precede the attention matmuls in PE order,
                # so the AllReduce has the full ~27us Q window to complete.
                attn_sb = [sb.tile([P, SH], F16, tag="at", bufs=8, name=f"attn{m}")
                           for m in range(MT)]
                qpend = []
                for m in range(MT):
                    qp0 = psp.tile([P, NB], F32, tag="mm", bufs=6, name="qp0")
                    qp1 = psp.tile([P, NB], F32, tag="mm", bufs=6, name="qp1")
                    for kt in range(KT):
                        for nb, qp in ((0, qp0), (1, qp1)):
                            nc.tensor.matmul(
                                qp[:],
                                lhsT=wq_sb[kt][:, m * P:(m + 1) * P],
                                rhs=xT_sb[kt][:, nb * NB:(nb + 1) * NB],
                                start=(kt == 0), stop=(kt == KT - 1),
                            )
                    for nb, qp in ((0, qp0), (1, qp1)):
                        qt = sb.tile([P, NB], F16, tag="qt", bufs=17, name="qt")
                        nc.scalar.activation(qt[:], qp[:], AF.Identity,
                                             bias=bq_sb[:, m:m + 1], scale=1.0)
                        qpend.append((m, nb, qt))
                # nb-major attention order: the 8 nb=0 evictions complete
                # first, so MLP1's nb=0 sweep (which contracts over all 8
                # pairs at one sequence block) starts ~5us earlier.
                qpend.sort(key=lambda t: (t[1], t[0]))
                for m, nb, qt in qpend:
                    # qt partitions are [head 2m d | head 2m+1 d], matching
                    # the kv pair-block rows; with the junk quadrants zeroed
                    # one 128-contraction matmul yields both heads' attn in
                    # the right partition layout.
                    nsl = slice(nb * NB, (nb + 1) * NB)
                    ap_ = psp.tile([P, NB], F32, tag="mm", bufs=6, name="ap_")
                    nc.tensor.matmul(
                        ap_[:],
                        lhsT=kv_red[:, m * 128:(m + 1) * 128],
                        rhs=qt[:], start=True, stop=True)
                    nc.vector.tensor_copy(attn_sb[m][:, nsl], ap_[:])

                if phases <= 2:
                    for m in range(MT):
                        for nb in range(NBLK):
                            nsl = slice(nb * NB, (nb + 1) * NB)
                            nc.gpsimd.dma_start(out=outT[m * P:(m + 1) * P, nsl],
                                                in_=attn_sb[m][:, nsl])
                    return

                # ---- phase 5: MLP hidden with fused exact GELU ----
                h1_sb = [sb.tile([P, SH], F16, tag="h1", bufs=8, name=f"h1{m}")
                         for m in range(MT)]
                for nb in range(NBLK):
                    nsl = slice(nb * NB, (nb + 1) * NB)
                    for m in range(MT):
                        hp = psp.tile([P, NB], F32, tag="mm", bufs=6, name="hp")
                        for kt in range(KT):
                            nc.tensor.matmul(
                                hp[:],
                                lhsT=w1_sb[kt][:, m * P:(m + 1) * P],
                                rhs=attn_sb[kt][:, nsl],
                                start=(kt == 0), stop=(kt == KT - 1),
                            )
                        nc.scalar.activation(h1_sb[m][:, nsl], hp[:],
                                             AF.Gelu if gelu else AF.Identity,
                                             bias=b1_sb[:, m:m + 1], scale=1.0)

                if phases <= 3:
                    for m in range(MT):
                        for nb in range(NBLK):
                            nsl = slice(nb * NB, (nb + 1) * NB)
                            nc.gpsimd.dma_start(out=outT[m * P:(m + 1) * P, nsl],
                                                in_=h1_sb[m][:, nsl])
                    return

                # fold the attention output into the residual so the MLP2
                # eviction is a single DVE op per block
                for m in range(MT):
                    nc.vector.tensor_add(xT_sb[m][:], xT_sb[m][:],
                                         attn_sb[m][:])

                # ---- phase 6: MLP out + residuals; stores on Pool queue ----
                for nb in range(NBLK):
                    nsl = slice(nb * NB, (nb + 1) * NB)
                    for m in range(MT):
                        op = psp.tile([P, NB], F32, tag="mm", bufs=6, name="op")
                        for kt in range(KT):
                            nc.tensor.matmul(
                                op[:],
                                lhsT=w2_sb[kt][:, m * P:(m + 1) * P],
                                rhs=h1_sb[kt][:, nsl],
                                start=(kt == 0), stop=(kt == KT - 1),
                            )
                        stg = sb.tile([P, NB], ODT, tag="stg", bufs=3, name="stg")
                        # stg = (op + b2) + (x + attn)   [attn pre-folded into xT]
                        nc.vector.scalar_tensor_tensor(
                            stg[:], op[:], b2_sb[:, m:m + 1],
                            xT_sb[m][:, nsl], op0=ALU.add, op1=ALU.add)
                        nc.gpsimd.dma_start(
                            out=outT[m * P:(m + 1) * P, nsl], in_=stg[:])

            # Straight-line unroll for timing runs (collectives cannot sit
            # inside a hardware For_i loop on this execution path).
            for _ in range(loop_r):
                body()

    nc.compile()
    return nc


def _get_program(with_bkv: bool, loop_r: int = 1, use_cc: bool = True,
                 phases: int = 4, mmx: int = 1):
    key = (with_bkv, loop_r, use_cc, phases, mmx)
    if key not in _CACHE:
        _CACHE[key] = _build(with_bkv, loop_r, use_cc=use_cc, phases=phases,
                             mmx=mmx)
    return _CACHE[key]


def _pack_inputs(x, data, Wq, bq, Wk, bk, Wv, bv, W1, b1, W2, b2, with_bkv):
    import ml_dtypes
    f32 = np.float32
    f16 = {"f16": np.float16, "bf16": ml_dtypes.bfloat16,
           "f32r": np.float32}[GDT]
    wq_s = np.ascontiguousarray(np.asarray(Wq, f32) * f32(SCALE), dtype=f32).astype(f16)
    wk_c = np.asarray(Wk, f32).astype(f16)
    wv_c = np.asarray(Wv, f32).astype(f16)
    w1_c = np.asarray(W1, f32).astype(f16)
    w2_c = np.asarray(W2, f32).astype(f16)
    bqt = np.ascontiguousarray((np.asarray(bq, f32) * f32(SCALE)).reshape(MT, P).T)
    b1t = np.ascontiguousarray(np.asarray(b1, f32).reshape(MT, P).T)
    b2t = np.ascontiguousarray(np.asarray(b2, f32).reshape(MT, P).T)
    in_maps = []
    for c in range(8):
        b_, j = divmod(c, 2)
        m = {
            "xT": np.ascontiguousarray(np.asarray(x, f32)[b_, j * SH:(j + 1) * SH, :].T).astype(f16),
            "dT": np.ascontiguousarray(np.asarray(data, f32)[b_, j * SH:(j + 1) * SH, :].T).astype(f16),
            "wq": wq_s, "wk": wk_c, "wv": wv_c, "w1": w1_c, "w2": w2_c,
            "bqt": bqt, "b1t": b1t, "b2t": b2t,
        }
        if with_bkv:
            m["bkr"] = np.asarray(bk, f32).reshape(1, C).astype(f16)
            m["bvr"] = np.asarray(bv, f32).reshape(1, C).astype(f16)
        in_maps.append(m)
    return in_maps


def run_on_hw(inputs, loop_r: int = 1, trace: bool = False):
    """Run the SPMD program; returns BassKernelResults."""
    from concourse.bass_utils import run_bass_kernel_spmd

    with_bkv = bool(
        np.any(np.asarray(inputs["bk"])) or np.any(np.asarray(inputs["bv"]))
    )
    nc = _get_program(with_bkv, loop_r)
    in_maps = _pack_inputs(
        inputs["x"], inputs["data"], inputs["Wq"], inputs["bq"], inputs["Wk"],
        inputs["bk"], inputs["Wv"], inputs["bv"], inputs["W1"], inputs["b1"],
        inputs["W2"], inputs["b2"], with_bkv,
    )
    res = run_bass_kernel_spmd(nc, in_maps, list(range(8)), trace=trace)
    return res


def kernel(**inputs) -> np.ndarray:
    res = run_on_hw(inputs, loop_r=1)
    out = np.empty((B, S, C), dtype=np.float32)
    for c in range(8):
        b_, j = divmod(c, 2)
        out[b_, j * SH:(j + 1) * SH, :] = res.results[c]["outT"].T.astype(np.float32)
    return out



# revision 2
# speedup vs baseline: 1.0974x; 1.0974x over previous
"""Trainium2 Bass kernel for nn_Attention_56736517980393.

Reference computation (B=4, S=2048, C=1024, H=16 heads, D=64, MLP hidden 1024):
    q = (x @ Wq + bq) * D**-0.5          per-head [B,H,S,D]
    k = data @ Wk + bk ; v = data @ Wv + bv
    kv[b,h] = k^T @ v                     [D,D]   (no softmax -> associative form)
    attn = q @ kv                         [B,S,C]
    out = x + attn + gelu(attn @ W1 + b1) @ W2 + b2

Sharding: 8 cores = (batch b in 0..3) x (sequence half j in 0..1).
Each core computes K/V projections for its (b, j) sequence half, a partial
kv (reduced over its half), AllReduces kv with its pair core, then computes
Q / attn / MLP / residual for its half.  Activations are kept feature-major
(transposed, [C, S]) on chip so that biases are per-partition and no on-chip
transposes are needed; K and V are produced sequence-major for the kv matmul
by using the activation tile as the stationary matmul operand.

Precision: the K and V projections run in fp8e4 (TRN FP8_EXP4, max 240)
with perf_mode=DoubleRow -- 256-deep contraction per pass, ~1.8x the bf16
column rate -- with power-of-2 pre-scales (data x32, weights x1024) undone
exactly at the PSUM eviction (x 2^-15).  Everything else stays bf16.
End-to-end max-rel error vs the fp32 reference is ~1.9e-2 against the
harness budget of 2e-2 (deterministic inputs; fp8 numerics verified
bit-faithful to the ml_dtypes emulation).  Set F8=k for K-only (~1.2e-2)
or F8= (empty) for the all-bf16 variant (~5.3e-3).

The kv matmuls (128-col, one stationary each) are LDWEIGHTS-bound when
emitted as a block (107ns load vs 53ns stream); they are interleaved into
the following projection's matmul stream so each load hides under a
512-col matmul.

DMA queues: bulk loads ride the SP queue in exact consumption order;
the kv AllReduce staging and output stores ride the GpSimd SWDGE (the
Pool sequencer is otherwise idle).  Phase-1 PSUM evictions split across
Act and DVE.
"""

import os
import numpy as np

GDT = os.environ.get("GDT", "bf16")
F8 = os.environ.get("F8", "kv")          # "kv" | "k" | ""

B, S, C, H, D = 4, 2048, 1024, 16, 64
SH = S // 2          # sequence rows per core
SCALE = D ** -0.5
P = 128              # SBUF partitions
NB = 512             # matmul moving free-dim block (one PSUM bank of fp32)
KT = C // P          # 8 contraction tiles
MT = C // P          # 8 output-feature tiles
NBLK = SH // NB      # 2 sequence blocks
DT8 = C // 256       # 4 DoubleRow contraction tiles
SD8 = 32.0           # fp8 pre-scale on data
SW8 = 1024.0         # fp8 pre-scale on Wk/Wv
SINV = 1.0 / (SD8 * SW8)

_CACHE = {}


def _build(with_bkv: bool, loop_r: int = 1, n_cores: int = 8, use_cc: bool = True,
           phases: int = 4, f8: str = F8, gelu: bool = True):
    import concourse.bacc as bacc
    import concourse.mybir as mybir
    from concourse.tile import TileContext

    assert not (with_bkv and f8), "fp8 path keeps biases unsupported; use F8="
    F32 = mybir.dt.float32
    F16 = {"f16": mybir.dt.float16, "bf16": mybir.dt.bfloat16,
           "f32r": mybir.dt.float32r}[GDT]
    F8E4 = mybir.dt.float8e4
    DRM = mybir.MatmulPerfMode.DoubleRow
    ODT = mybir.dt.float16
    AF = mybir.ActivationFunctionType
    ALU = mybir.AluOpType

    nc = bacc.Bacc(
        trn_type="TRN2", target_bir_lowering=False, debug=False, num_devices=n_cores
    )

    xT = nc.dram_tensor("xT", [C, SH], F16, kind="ExternalInput").ap()
    if f8 != "kv":
        dT = nc.dram_tensor("dT", [C, SH], F16, kind="ExternalInput").ap()
    if f8:
        dT8 = nc.dram_tensor("dT8", [DT8 * P, 2, SH], F8E4,
                             kind="ExternalInput").ap()
        wk8 = nc.dram_tensor("wk8", [DT8 * P, 2, C], F8E4,
                             kind="ExternalInput").ap()
    else:
        wk = nc.dram_tensor("wk", [C, C], F16, kind="ExternalInput").ap()
    if f8 == "kv":
        wv8 = nc.dram_tensor("wv8", [DT8 * P, 2, C], F8E4,
                             kind="ExternalInput").ap()
    else:
        wv = nc.dram_tensor("wv", [C, C], F16, kind="ExternalInput").ap()
    wq = nc.dram_tensor("wq", [C, C], F16, kind="ExternalInput").ap()
    w1 = nc.dram_tensor("w1", [C, C], F16, kind="ExternalInput").ap()
    w2 = nc.dram_tensor("w2", [C, C], F16, kind="ExternalInput").ap()
    # feature-major biases: [128, 8] so that column m is the per-partition
    # bias for feature tile m
    bqt = nc.dram_tensor("bqt", [P, MT], F32, kind="ExternalInput").ap()
    b1t = nc.dram_tensor("b1t", [P, MT], F32, kind="ExternalInput").ap()
    b2t = nc.dram_tensor("b2t", [P, MT], F32, kind="ExternalInput").ap()
    if with_bkv:
        bkr = nc.dram_tensor("bkr", [1, C], F16, kind="ExternalInput").ap()
        bvr = nc.dram_tensor("bvr", [1, C], F16, kind="ExternalInput").ap()
    outT = nc.dram_tensor("outT", [C, SH], ODT, kind="ExternalOutput").ap()

    groups = [[i, i + 1] for i in range(0, n_cores, 2)]

    with TileContext(nc) as tc:
        with tc.tile_pool(name="sb", bufs=1) as sb, \
             tc.tile_pool(name="ps", bufs=1, space="PSUM") as psp, \
             tc.tile_pool(name="dr", bufs=1, space="DRAM") as drp:

            # One-time act-table preload: the gelu_and_others set also holds
            # Identity and Copy, so every later Act op is served without a
            # 1.3us table swap.
            if gelu:
                warm = sb.tile([1, 8], F32, tag="warm", bufs=1, name="warm")
                nc.vector.memset(warm[:], 0.0)
                nc.scalar.activation(warm[:], warm[:], AF.Gelu, scale=1.0)

            def body(it=None):
                # ---- SBUF tiles ----
                if f8:
                    dT8_sb = [sb.tile([P, 2, SH], F8E4, tag="d8", bufs=DT8,
                                      name=f"dT8{t}") for t in range(DT8)]
                    wk8_sb = [sb.tile([P, 2, C], F8E4, tag="w8",
                                      bufs=(2 * DT8 if f8 == "kv" else DT8),
                                      name=f"wk8{t}") for t in range(DT8)]
                if f8 == "kv":
                    wv8_sb = [sb.tile([P, 2, C], F8E4, tag="w8", bufs=2 * DT8,
                                      name=f"wv8{t}") for t in range(DT8)]
                else:
                    wv_sb = [sb.tile([P, C], F16, tag="wt", bufs=40,
                                     name=f"wv{i}") for i in range(KT)]
                    dT_sb = [sb.tile([P, SH], F16, tag="dh", bufs=8,
                                     name=f"dT{i}") for i in range(KT)]
                if not f8:
                    wk_sb = [sb.tile([P, C], F16, tag="wt", bufs=40,
                                     name=f"wk{i}") for i in range(KT)]
                xT_sb = [sb.tile([P, SH], F16, tag="xa", bufs=8, name=f"xT{i}")
                         for i in range(KT)]
                # ---- SP load queue: strict phase-1 consumption order, then
                # next-phase prefetch.  Everything is resident well before use.
                if f8 == "kv":
                    for t in range(DT8):
                        nc.sync.dma_start(out=dT8_sb[t][:, :, 0:P],
                                          in_=dT8[t * P:(t + 1) * P, :, 0:P])
                        nc.sync.dma_start(out=wk8_sb[t][:], in_=wk8[t * P:(t + 1) * P, :, :])
                    for t in range(DT8):
                        nc.sync.dma_start(out=dT8_sb[t][:, :, P:SH],
                                          in_=dT8[t * P:(t + 1) * P, :, P:SH])
                    for t in range(DT8):
                        nc.sync.dma_start(out=wv8_sb[t][:], in_=wv8[t * P:(t + 1) * P, :, :])
                elif f8 == "k":
                    for t in range(DT8):
                        nc.sync.dma_start(out=dT8_sb[t][:, :, 0:P],
                                          in_=dT8[t * P:(t + 1) * P, :, 0:P])
                        nc.sync.dma_start(out=wk8_sb[t][:], in_=wk8[t * P:(t + 1) * P, :, :])
                    for i in range(KT):
                        nc.sync.dma_start(out=dT_sb[i][:, 0:P], in_=dT[i * P:(i + 1) * P, 0:P])
                    for t in range(DT8):
                        nc.sync.dma_start(out=dT8_sb[t][:, :, P:SH],
                                          in_=dT8[t * P:(t + 1) * P, :, P:SH])
                    for i in range(KT):
                        nc.sync.dma_start(out=dT_sb[i][:, P:SH], in_=dT[i * P:(i + 1) * P, P:SH])
                    for i in range(KT):
                        nc.sync.dma_start(out=wv_sb[i][:], in_=wv[i * P:(i + 1) * P, :])
                else:
                    for i in range(KT):
                        nc.sync.dma_start(out=dT_sb[i][:, 0:P], in_=dT[i * P:(i + 1) * P, 0:P])
                        nc.sync.dma_start(out=wk_sb[i][:], in_=wk[i * P:(i + 1) * P, :])
                    for i in range(KT):
                        nc.sync.dma_start(out=dT_sb[i][:, P:SH], in_=dT[i * P:(i + 1) * P, P:SH])
                    for i in range(KT):
                        nc.sync.dma_start(out=wv_sb[i][:], in_=wv[i * P:(i + 1) * P, :])
                bq_sb = sb.tile([P, MT], F32, tag="bias", bufs=3, name="bq_sb")
                b1_sb = sb.tile([P, MT], F32, tag="bias", bufs=3, name="b1_sb")
                b2_sb = sb.tile([P, MT], F32, tag="bias", bufs=3, name="b2_sb")
                nc.sync.dma_start(out=bq_sb[:], in_=bqt[:])
                nc.sync.dma_start(out=b1_sb[:], in_=b1t[:])
                nc.sync.dma_start(out=b2_sb[:], in_=b2t[:])
                if with_bkv:
                    bk_sb = sb.tile([1, C], F16, tag="brow", bufs=2, name="bk_sb")
                    bv_sb = sb.tile([1, C], F16, tag="brow", bufs=2, name="bv_sb")
                    ones = sb.tile([1, P], F16, tag="ones", bufs=1, name="ones")
                    nc.sync.dma_start(out=bk_sb[:], in_=bkr[:])
                    nc.sync.dma_start(out=bv_sb[:], in_=bvr[:])
                    nc.vector.memset(ones[:], 1.0)
                wq_sb = [sb.tile([P, C], F16, tag="wt", bufs=40, name=f"wq{i}")
                         for i in range(KT)]
                w1_sb = [sb.tile([P, C], F16, tag="wt", bufs=40, name=f"w1{i}")
                         for i in range(KT)]
                w2_sb = [sb.tile([P, C], F16, tag="wt", bufs=40, name=f"w2{i}")
                         for i in range(KT)]
                for i in range(KT):
                    nc.sync.dma_start(out=xT_sb[i][:], in_=xT[i * P:(i + 1) * P, :])
                    nc.sync.dma_start(out=wq_sb[i][:], in_=wq[i * P:(i + 1) * P, :])
                for i in range(KT):
                    nc.sync.dma_start(out=w1_sb[i][:], in_=w1[i * P:(i + 1) * P, :])
                for i in range(KT):
                    nc.sync.dma_start(out=w2_sb[i][:], in_=w2[i * P:(i + 1) * P, :])

                # ---- phase 1: K/V (sequence-major) and kv partial ----
                # Every stationary serves both feature halves (paired matmuls
                # into two PSUM banks) so the PE loads each stationary once
                # per two matmuls.  kv pairs two heads per [128,128] k
                # stationary; the cross-head half of each product lands in
                # the unused partition half of its bank (bank A even heads at
                # partitions 0:64, bank B odd heads at 64:128, pair i at
                # free i*128).  kv matmuls are interleaved into the next
                # projection's stream via `extras` so their LDWEIGHTS hide.
                kv_ps_a = psp.tile([P, NB], F32, tag="acc", bufs=2, name="kv_ps_a")
                kv_ps_b = psp.tile([P, NB], F32, tag="acc", bufs=2, name="kv_ps_b")

                def proj_seq(w_sb, brow, st, nm, extras=None):
                    ssl = slice(st * P, (st + 1) * P)
                    t = sb.tile([P, C], F16, tag="kvs", bufs=5, name=nm)
                    pk0 = psp.tile([P, NB], F32, tag="mm", bufs=6, name="pk0")
                    pk1 = psp.tile([P, NB], F32, tag="mm", bufs=6, name="pk1")
                    ex = list(extras) if extras else []
                    for kt in range(KT):
                        for nb, pk in ((0, pk0), (1, pk1)):
                            nc.tensor.matmul(
                                pk[:],
                                lhsT=dT_sb[kt][:, ssl],
                                rhs=w_sb[kt][:, nb * NB:(nb + 1) * NB],
                                start=(kt == 0),
                                stop=(kt == KT - 1 and not with_bkv),
                            )
                        if ex:
                            ex.pop(0)()
                    if with_bkv:
                        nc.tensor.matmul(pk0[:], lhsT=ones[:],
                                         rhs=brow[:, 0:NB],
                                         start=False, stop=True)
                        nc.tensor.matmul(pk1[:], lhsT=ones[:],
                                         rhs=brow[:, NB:C],
                                         start=False, stop=True)
                    nc.scalar.copy(t[:, 0:NB], pk0[:])
                    nc.vector.tensor_copy(t[:, NB:C], pk1[:])
                    return t

                def proj_seq_f8(w8_sb_, st, nm, extras=None):
                    # DoubleRow: contraction 256 per pass, stationary is the
                    # fp8 data slice [128,2,128], moving the fp8 weight
                    # [128,2,512].  Evictions undo the 2^15 pre-scale.
                    t = sb.tile([P, C], F16, tag="kvs", bufs=5, name=nm)
                    pk0 = psp.tile([P, NB], F32, tag="mm", bufs=6, name="pk0")
                    pk1 = psp.tile([P, NB], F32, tag="mm", bufs=6, name="pk1")
                    ex = list(extras) if extras else []
                    for tt in range(DT8):
                        for nb, pk in ((0, pk0), (1, pk1)):
                            nc.tensor.matmul(
                                pk[:],
                                lhsT=dT8_sb[tt][:, :, st * P:(st + 1) * P],
                                rhs=w8_sb_[tt][:, :, nb * NB:(nb + 1) * NB],
                                start=(tt == 0),
                                stop=(tt == DT8 - 1),
                                perf_mode=DRM,
                            )
                        while ex:
                            ex.pop(0)()
                    nc.scalar.activation(t[:, 0:NB], pk0[:], AF.Copy, scale=SINV)
                    nc.vector.tensor_scalar_mul(t[:, NB:C], pk1[:], SINV)
                    return t

                def kv_thunks(k_t, v_t, st):
                    # pair i: full [128,128] product of the pair's k and v
                    # slices.  Quadrants [0:64,0:64] and [64:128,64:128] are
                    # the two heads' kv; the off-diagonal quadrants are
                    # cross-head junk, zeroed after the AllReduce.
                    def mk(i):
                        def go():
                            tgt = kv_ps_a if i < 4 else kv_ps_b
                            fo = (i % 4) * 128
                            nc.tensor.matmul(
                                tgt[:, fo:fo + 128],
                                lhsT=k_t[:, i * 128:(i + 1) * 128],
                                rhs=v_t[:, i * 128:(i + 1) * 128],
                                start=(st == 0 and i % 4 == 0),
                                stop=(st == MT - 1 and i % 4 == 3),
                            )
                        return go
                    return [mk(i) for i in range(8)]

                def PROJ_K(st, extras=None):
                    if f8:
                        return proj_seq_f8(wk8_sb, st, "k_t", extras)
                    return proj_seq(wk_sb, bk_sb if with_bkv else None, st,
                                    "k_t", extras)

                def PROJ_V(st, extras=None):
                    if f8 == "kv":
                        return proj_seq_f8(wv8_sb, st, "v_t", extras)
                    return proj_seq(wv_sb, bv_sb if with_bkv else None, st,
                                    "v_t", extras)

                pend = None
                for st in range(MT):
                    k_t = PROJ_K(st)
                    v_t = PROJ_V(st, extras=kv_thunks(*pend) if pend else None)
                    pend = (k_t, v_t, st)
                kv7 = kv_thunks(*pend)

                # ---- phase 2: AllReduce kv across the pair ----
                # kv_sb [128, 1024]: pair i at cols i*128 (junk rides along;
                # junk+junk is still junk and gets zeroed after the reduce).
                kv_sb = sb.tile([P, C], F16, tag="kvio", bufs=2, name="kv_sb")
                kv_in = drp.tile([P, C], F16, tag="dri", bufs=1, name="kv_in")
                kv_out = drp.tile([P, C], F16, tag="dro", bufs=1, name="kv_out")
                kv_red = sb.tile([P, C], F16, tag="kvr", bufs=2, name="kv_red")

                def kv_exchange():
                    nc.vector.tensor_copy(kv_sb[:, 0:NB], kv_ps_a[:])
                    nc.vector.tensor_copy(kv_sb[:, NB:C], kv_ps_b[:])
                    nc.gpsimd.dma_start(out=kv_in[:], in_=kv_sb[:])
                    if n_cores == 1 or not use_cc:
                        # single-core analysis build: stand in for the AllReduce
                        nc.gpsimd.dma_start(out=kv_out[:], in_=kv_in[:])
                    else:
                        nc.gpsimd.collective_compute(
                            "AllReduce", ALU.add, replica_groups=groups,
                            ins=[kv_in.opt()], outs=[kv_out.opt()],
                        )
                    nc.gpsimd.dma_start(out=kv_red[:], in_=kv_out[:])
                    # zero the cross-head quadrants so attention can contract
                    # over the full 128 partitions in one matmul per pair
                    for i in range(8):
                        co = i * 128
                        nc.vector.memset(kv_red[64:128, co:co + 64], 0.0)
                        nc.vector.memset(kv_red[0:64, co + 64:co + 128], 0.0)

                if phases <= 1:
                    for th in kv7:
                        th()
                    kv_exchange()
                    stg0 = sb.tile([P, NB], ODT, tag="stg", bufs=3, name="stg0")
                    nc.vector.tensor_copy(stg0[:], kv_red[:, 0:NB])
                    for m in range(MT):
                        for nb in range(NBLK):
                            nsl = slice(nb * NB, (nb + 1) * NB)
                            nc.gpsimd.dma_start(out=outT[m * P:(m + 1) * P, nsl],
                                                in_=stg0[:])
                    return

                # ---- phase 3: Q projection (all blocks), then attention ----
                # All 16 Q blocks precede the attention matmuls in PE order,
                # so the AllReduce has the full ~27us Q window to complete.
                # The last sequence block's kv matmuls ride in the first Q
                # blocks (from the 3rd stationary on, so the v_t eviction has
                # time to land).
                attn_sb = [sb.tile([P, SH], F16, tag="at", bufs=8, name=f"attn{m}")
                           for m in range(MT)]
                exchanged = []

                def q_extra():
                    if kv7:
                        kv7.pop(0)()
                    elif not exchanged:
                        exchanged.append(1)
                        kv_exchange()

                qpend = []
                nstat = 0
                for m in range(MT):
                    qp0 = psp.tile([P, NB], F32, tag="mm", bufs=6, name="qp0")
                    qp1 = psp.tile([P, NB], F32, tag="mm", bufs=6, name="qp1")
                    for kt in range(KT):
                        for nb, qp in ((0, qp0), (1, qp1)):
                            nc.tensor.matmul(
                                qp[:],
                                lhsT=wq_sb[kt][:, m * P:(m + 1) * P],
                                rhs=xT_sb[kt][:, nb * NB:(nb + 1) * NB],
                                start=(kt == 0), stop=(kt == KT - 1),
                            )
                        nstat += 1
                        if nstat >= 3:
                            q_extra()
                    for nb, qp in ((0, qp0), (1, qp1)):
                        qt = sb.tile([P, NB], F16, tag="qt", bufs=17, name="qt")
                        nc.scalar.activation(qt[:], qp[:], AF.Identity,
                                             bias=bq_sb[:, m:m + 1], scale=1.0)
                        qpend.append((m, nb, qt))
                while kv7:
                    kv7.pop(0)()
                if not exchanged:
                    kv_exchange()
                # nb-major attention order: the 8 nb=0 evictions complete
                # first, so MLP1's nb=0 sweep (which contracts over all 8
                # pairs at one sequence block) starts ~5us earlier.
                qpend.sort(key=lambda t: (t[1], t[0]))
                for m, nb, qt in qpend:
                    # qt partitions are [head 2m d | head 2m+1 d], matching
                    # the kv pair-block rows; with the junk quadrants zeroed
                    # one 128-contraction matmul yields both heads' attn in
                    # the right partition layout.
                    nsl = slice(nb * NB, (nb + 1) * NB)
                    ap_ = psp.tile([P, NB], F32, tag="mm", bufs=6, name="ap_")
                    nc.tensor.matmul(
                        ap_[:],
                        lhsT=kv_red[:, m * 128:(m + 1) * 128],
                        rhs=qt[:], start=True, stop=True)
                    nc.vector.tensor_copy(attn_sb[m][:, nsl], ap_[:])

                if phases <= 2:
                    for m in range(MT):
                        for nb in range(NBLK):
                            nsl = slice(nb * NB, (nb + 1) * NB)
                            nc.gpsimd.dma_start(out=outT[m * P:(m + 1) * P, nsl],
                                                in_=attn_sb[m][:, nsl])
                    return

                # ---- phase 5: MLP hidden with fused exact GELU ----
                h1_sb = [sb.tile([P, SH], F16, tag="h1", bufs=8, name=f"h1{m}")
                         for m in range(MT)]
                for nb in range(NBLK):
                    nsl = slice(nb * NB, (nb + 1) * NB)
                    for m in range(MT):
                        hp = psp.tile([P, NB], F32, tag="mm", bufs=6, name="hp")
                        for kt in range(KT):
                            nc.tensor.matmul(
                                hp[:],
                                lhsT=w1_sb[kt][:, m * P:(m + 1) * P],
                                rhs=attn_sb[kt][:, nsl],
                                start=(kt == 0), stop=(kt == KT - 1),
                            )
                        nc.scalar.activation(h1_sb[m][:, nsl], hp[:],
                                             AF.Gelu if gelu else AF.Identity,
                                             bias=b1_sb[:, m:m + 1], scale=1.0)

                if phases <= 3:
                    for m in range(MT):
                        for nb in range(NBLK):
                            nsl = slice(nb * NB, (nb + 1) * NB)
                            nc.gpsimd.dma_start(out=outT[m * P:(m + 1) * P, nsl],
                                                in_=h1_sb[m][:, nsl])
                    return

                # fold the attention output into the residual so the MLP2
                # eviction is a single DVE op per block
                for m in range(MT):
                    nc.vector.tensor_add(xT_sb[m][:], xT_sb[m][:],
                                         attn_sb[m][:])

                # ---- phase 6: MLP out + residuals; stores on Pool queue ----
                for nb in range(NBLK):
                    nsl = slice(nb * NB, (nb + 1) * NB)
                    for m in range(MT):
                        op = psp.tile([P, NB], F32, tag="mm", bufs=6, name="op")
                        for kt in range(KT):
                            nc.tensor.matmul(
                                op[:],
                                lhsT=w2_sb[kt][:, m * P:(m + 1) * P],
                                rhs=h1_sb[kt][:, nsl],
                                start=(kt == 0), stop=(kt == KT - 1),
                            )
                        stg = sb.tile([P, NB], ODT, tag="stg", bufs=3, name="stg")
                        # stg = (op + b2) + (x + attn)   [attn pre-folded into xT]
                        nc.vector.scalar_tensor_tensor(
                            stg[:], op[:], b2_sb[:, m:m + 1],
                            xT_sb[m][:, nsl], op0=ALU.add, op1=ALU.add)
                        nc.gpsimd.dma_start(
                            out=outT[m * P:(m + 1) * P, nsl], in_=stg[:])

            # Straight-line unroll for timing runs (collectives cannot sit
            # inside a hardware For_i loop on this execution path).
            for _ in range(loop_r):
                body()

    nc.compile()
    return nc


def _get_program(with_bkv: bool, loop_r: int = 1, use_cc: bool = True,
                 phases: int = 4, f8: str = F8):
    key = (with_bkv, loop_r, use_cc, phases, f8)
    if key not in _CACHE:
        _CACHE[key] = _build(with_bkv, loop_r, use_cc=use_cc, phases=phases,
                             f8=f8)
    return _CACHE[key]


def _pack3(a8):
    """[C, w] (contraction-major) -> [DT8*128, 2, w] DoubleRow interleave."""
    cw = a8.shape[1]
    return np.ascontiguousarray(
        a8.reshape(DT8, 2, P, cw).transpose(0, 2, 1, 3).reshape(DT8 * P, 2, cw)
    )


def _pack_inputs(x, data, Wq, bq, Wk, bk, Wv, bv, W1, b1, W2, b2, with_bkv,
                 f8: str = F8):
    import ml_dtypes
    f32 = np.float32
    f16 = {"f16": np.float16, "bf16": ml_dtypes.bfloat16,
           "f32r": np.float32}[GDT]
    f8dt = ml_dtypes.float8_e4m3
    wq_s = np.ascontiguousarray(np.asarray(Wq, f32) * f32(SCALE), dtype=f32).astype(f16)
    w1_c = np.asarray(W1, f32).astype(f16)
    w2_c = np.asarray(W2, f32).astype(f16)
    if f8:
        wkf = np.asarray(Wk, f32) * f32(SW8)
        assert np.abs(wkf).max() < 240.0, "Wk fp8 overflow"
        wk8_c = _pack3(wkf.astype(f8dt))
    else:
        wk_c = np.asarray(Wk, f32).astype(f16)
    if f8 == "kv":
        wvf = np.asarray(Wv, f32) * f32(SW8)
        assert np.abs(wvf).max() < 240.0, "Wv fp8 overflow"
        wv8_c = _pack3(wvf.astype(f8dt))
    else:
        wv_c = np.asarray(Wv, f32).astype(f16)
    bqt = np.ascontiguousarray((np.asarray(bq, f32) * f32(SCALE)).reshape(MT, P).T)
    b1t = np.ascontiguousarray(np.asarray(b1, f32).reshape(MT, P).T)
    b2t = np.ascontiguousarray(np.asarray(b2, f32).reshape(MT, P).T)
    in_maps = []
    for c in range(8):
        b_, j = divmod(c, 2)
        dTf = np.ascontiguousarray(np.asarray(data, f32)[b_, j * SH:(j + 1) * SH, :].T)
        m = {
            "xT": np.ascontiguousarray(np.asarray(x, f32)[b_, j * SH:(j + 1) * SH, :].T).astype(f16),
            "wq": wq_s, "w1": w1_c, "w2": w2_c,
            "bqt": bqt, "b1t": b1t, "b2t": b2t,
        }
        if f8:
            d8 = dTf * f32(SD8)
            assert np.abs(d8).max() < 240.0, "data fp8 overflow"
            m["dT8"] = _pack3(d8.astype(f8dt))
            m["wk8"] = wk8_c
        else:
            m["wk"] = wk_c
        if f8 == "kv":
            m["wv8"] = wv8_c
        else:
            m["wv"] = wv_c
        if f8 != "kv":
            m["dT"] = dTf.astype(f16)
        if with_bkv:
            m["bkr"] = np.asarray(bk, f32).reshape(1, C).astype(f16)
            m["bvr"] = np.asarray(bv, f32).reshape(1, C).astype(f16)
        in_maps.append(m)
    return in_maps


def run_on_hw(inputs, loop_r: int = 1, trace: bool = False):
    """Run the SPMD program; returns BassKernelResults."""
    from concourse.bass_utils import run_bass_kernel_spmd

    with_bkv = bool(
        np.any(np.asarray(inputs["bk"])) or np.any(np.asarray(inputs["bv"]))
    )
    f8 = "" if with_bkv else F8
    nc = _get_program(with_bkv, loop_r, f8=f8)
    in_maps = _pack_inputs(
        inputs["x"], inputs["data"], inputs["Wq"], inputs["bq"], inputs["Wk"],
        inputs["bk"], inputs["Wv"], inputs["bv"], inputs["W1"], inputs["b1"],
        inputs["W2"], inputs["b2"], with_bkv, f8=f8,
    )
    res = run_bass_kernel_spmd(nc, in_maps, list(range(8)), trace=trace)
    return res


def kernel(**inputs) -> np.ndarray:
    res = run_on_hw(inputs, loop_r=1)
    out = np.empty((B, S, C), dtype=np.float32)
    for c in range(8):
        b_, j = divmod(c, 2)
        out[b_, j * SH:(j + 1) * SH, :] = res.results[c]["outT"].T.astype(np.float32)
    return out


# revision 3
# speedup vs baseline: 1.1502x; 1.0481x over previous
"""Trainium2 Bass kernel for nn_Attention_56736517980393.

Reference computation (B=4, S=2048, C=1024, H=16 heads, D=64, MLP hidden 1024):
    q = (x @ Wq + bq) * D**-0.5          per-head [B,H,S,D]
    k = data @ Wk + bk ; v = data @ Wv + bv
    kv[b,h] = k^T @ v                     [D,D]   (no softmax -> associative form)
    attn = q @ kv                         [B,S,C]
    out = x + attn + gelu(attn @ W1 + b1) @ W2 + b2

Sharding: 8 cores = (batch b in 0..3) x (sequence half j in 0..1).
Each core computes K/V projections for its (b, j) sequence half, a partial
kv (reduced over its half), AllReduces kv with its pair core, then computes
Q / attn / MLP / residual for its half.  Activations are kept feature-major
(transposed, [C, S]) on chip so that biases are per-partition and no on-chip
transposes are needed; K and V are produced sequence-major for the kv matmul
by using the activation tile as the stationary matmul operand.

Precision: the K and V projections run in fp8e4 (TRN FP8_EXP4, max 240)
with perf_mode=DoubleRow -- 256-deep contraction per pass, ~1.8x the bf16
column rate -- with power-of-2 pre-scales (data x32, weights x1024) undone
exactly at the PSUM eviction (x 2^-15).  Everything else stays bf16.
End-to-end max-rel error vs the fp32 reference is ~1.9e-2 against the
harness budget of 2e-2 (deterministic inputs; fp8 numerics verified
bit-faithful to the ml_dtypes emulation).  Set F8=k for K-only (~1.2e-2)
or F8= (empty) for the all-bf16 variant (~5.3e-3).

The kv matmuls (128-col, one stationary each) are LDWEIGHTS-bound when
emitted as a block (107ns load vs 53ns stream); they are interleaved into
the following projection's matmul stream so each load hides under a
512-col matmul.

DMA queues: bulk loads ride the SP queue in exact consumption order;
the kv AllReduce staging and output stores ride the GpSimd SWDGE (the
Pool sequencer is otherwise idle).  Phase-1 PSUM evictions split across
Act and DVE.
"""

import os
import numpy as np

GDT = os.environ.get("GDT", "f16")
F8 = os.environ.get("F8", "kv")          # "kv" | "k" | ""

B, S, C, H, D = 4, 2048, 1024, 16, 64
SH = S // 2          # sequence rows per core
SCALE = D ** -0.5
P = 128              # SBUF partitions
NB = 512             # matmul moving free-dim block (one PSUM bank of fp32)
KT = C // P          # 8 contraction tiles
MT = C // P          # 8 output-feature tiles
NBLK = SH // NB      # 2 sequence blocks
DT8 = C // 256       # 4 DoubleRow contraction tiles
SD8 = 32.0           # fp8 pre-scale on data
SW8 = 1024.0         # fp8 pre-scale on Wk/Wv
SINV = 1.0 / (SD8 * SW8)

_CACHE = {}


def _build(with_bkv: bool, loop_r: int = 1, n_cores: int = 8, use_cc: bool = True,
           phases: int = 4, f8: str = F8, gelu: bool = True):
    import concourse.bacc as bacc
    import concourse.mybir as mybir
    from concourse.tile import TileContext

    assert not (with_bkv and f8), "fp8 path keeps biases unsupported; use F8="
    F32 = mybir.dt.float32
    F16 = {"f16": mybir.dt.float16, "bf16": mybir.dt.bfloat16,
           "f32r": mybir.dt.float32r}[GDT]
    F8E4 = mybir.dt.float8e4
    DRM = mybir.MatmulPerfMode.DoubleRow
    ODT = mybir.dt.float16
    AF = mybir.ActivationFunctionType
    ALU = mybir.AluOpType

    nc = bacc.Bacc(
        trn_type="TRN2", target_bir_lowering=False, debug=False, num_devices=n_cores
    )

    xT = nc.dram_tensor("xT", [C, SH], F16, kind="ExternalInput").ap()
    if f8 != "kv":
        dT = nc.dram_tensor("dT", [C, SH], F16, kind="ExternalInput").ap()
    if f8:
        dT8 = nc.dram_tensor("dT8", [DT8 * P, 2, SH], F8E4,
                             kind="ExternalInput").ap()
        wk8 = nc.dram_tensor("wk8", [DT8 * P, 2, C], F8E4,
                             kind="ExternalInput").ap()
    else:
        wk = nc.dram_tensor("wk", [C, C], F16, kind="ExternalInput").ap()
    if f8 == "kv":
        wv8 = nc.dram_tensor("wv8", [DT8 * P, 2, C], F8E4,
                             kind="ExternalInput").ap()
    else:
        wv = nc.dram_tensor("wv", [C, C], F16, kind="ExternalInput").ap()
    wq = nc.dram_tensor("wq", [C, C], F16, kind="ExternalInput").ap()
    w1 = nc.dram_tensor("w1", [C, C], F16, kind="ExternalInput").ap()
    w2 = nc.dram_tensor("w2", [C, C], F16, kind="ExternalInput").ap()
    # feature-major biases: [128, 8] so that column m is the per-partition
    # bias for feature tile m
    bqt = nc.dram_tensor("bqt", [P, MT], F32, kind="ExternalInput").ap()
    b1t = nc.dram_tensor("b1t", [P, MT], F32, kind="ExternalInput").ap()
    b2t = nc.dram_tensor("b2t", [P, MT], F32, kind="ExternalInput").ap()
    if with_bkv:
        bkr = nc.dram_tensor("bkr", [1, C], F16, kind="ExternalInput").ap()
        bvr = nc.dram_tensor("bvr", [1, C], F16, kind="ExternalInput").ap()
    outT = nc.dram_tensor("outT", [C, SH], ODT, kind="ExternalOutput").ap()

    groups = [[i, i + 1] for i in range(0, n_cores, 2)]

    with TileContext(nc) as tc:
        with tc.tile_pool(name="sb", bufs=1) as sb, \
             tc.tile_pool(name="ps", bufs=1, space="PSUM") as psp, \
             tc.tile_pool(name="dr", bufs=1, space="DRAM") as drp:

            # One-time act-table preload: the gelu_and_others set also holds
            # Identity and Copy, so every later Act op is served without a
            # 1.3us table swap.
            if gelu:
                warm = sb.tile([1, 8], F32, tag="warm", bufs=1, name="warm")
                nc.vector.memset(warm[:], 0.0)
                nc.scalar.activation(warm[:], warm[:], AF.Gelu, scale=1.0)

            def body(it=None):
                # ---- SBUF tiles ----
                if f8:
                    dT8_sb = [sb.tile([P, 2, SH], F8E4, tag="d8", bufs=DT8,
                                      name=f"dT8{t}") for t in range(DT8)]
                    wk8_sb = [sb.tile([P, 2, C], F8E4, tag="w8",
                                      bufs=(2 * DT8 if f8 == "kv" else DT8),
                                      name=f"wk8{t}") for t in range(DT8)]
                if f8 == "kv":
                    wv8_sb = [sb.tile([P, 2, C], F8E4, tag="w8", bufs=2 * DT8,
                                      name=f"wv8{t}") for t in range(DT8)]
                else:
                    wv_sb = [sb.tile([P, C], F16, tag="wt", bufs=40,
                                     name=f"wv{i}") for i in range(KT)]
                    dT_sb = [sb.tile([P, SH], F16, tag="dh", bufs=8,
                                     name=f"dT{i}") for i in range(KT)]
                if not f8:
                    wk_sb = [sb.tile([P, C], F16, tag="wt", bufs=40,
                                     name=f"wk{i}") for i in range(KT)]
                xT_sb = [sb.tile([P, SH], F16, tag="xa", bufs=8, name=f"xT{i}")
                         for i in range(KT)]
                # ---- SP load queue: strict phase-1 consumption order, then
                # next-phase prefetch.  Everything is resident well before use.
                if f8 == "kv":
                    for t in range(DT8):
                        nc.sync.dma_start(out=dT8_sb[t][:, :, 0:P],
                                          in_=dT8[t * P:(t + 1) * P, :, 0:P])
                        nc.sync.dma_start(out=wk8_sb[t][:], in_=wk8[t * P:(t + 1) * P, :, :])
                    for t in range(DT8):
                        nc.sync.dma_start(out=dT8_sb[t][:, :, P:SH],
                                          in_=dT8[t * P:(t + 1) * P, :, P:SH])
                    for t in range(DT8):
                        nc.sync.dma_start(out=wv8_sb[t][:], in_=wv8[t * P:(t + 1) * P, :, :])
                elif f8 == "k":
                    for t in range(DT8):
                        nc.sync.dma_start(out=dT8_sb[t][:, :, 0:P],
                                          in_=dT8[t * P:(t + 1) * P, :, 0:P])
                        nc.sync.dma_start(out=wk8_sb[t][:], in_=wk8[t * P:(t + 1) * P, :, :])
                    for i in range(KT):
                        nc.sync.dma_start(out=dT_sb[i][:, 0:P], in_=dT[i * P:(i + 1) * P, 0:P])
                    for t in range(DT8):
                        nc.sync.dma_start(out=dT8_sb[t][:, :, P:SH],
                                          in_=dT8[t * P:(t + 1) * P, :, P:SH])
                    for i in range(KT):
                        nc.sync.dma_start(out=dT_sb[i][:, P:SH], in_=dT[i * P:(i + 1) * P, P:SH])
                    for i in range(KT):
                        nc.sync.dma_start(out=wv_sb[i][:], in_=wv[i * P:(i + 1) * P, :])
                else:
                    for i in range(KT):
                        nc.sync.dma_start(out=dT_sb[i][:, 0:P], in_=dT[i * P:(i + 1) * P, 0:P])
                        nc.sync.dma_start(out=wk_sb[i][:], in_=wk[i * P:(i + 1) * P, :])
                    for i in range(KT):
                        nc.sync.dma_start(out=dT_sb[i][:, P:SH], in_=dT[i * P:(i + 1) * P, P:SH])
                    for i in range(KT):
                        nc.sync.dma_start(out=wv_sb[i][:], in_=wv[i * P:(i + 1) * P, :])
                bq_sb = sb.tile([P, MT], F32, tag="bias", bufs=3, name="bq_sb")
                b1_sb = sb.tile([P, MT], F32, tag="bias", bufs=3, name="b1_sb")
                b2_sb = sb.tile([P, MT], F32, tag="bias", bufs=3, name="b2_sb")
                nc.sync.dma_start(out=bq_sb[:], in_=bqt[:])
                nc.sync.dma_start(out=b1_sb[:], in_=b1t[:])
                nc.sync.dma_start(out=b2_sb[:], in_=b2t[:])
                if with_bkv:
                    bk_sb = sb.tile([1, C], F16, tag="brow", bufs=2, name="bk_sb")
                    bv_sb = sb.tile([1, C], F16, tag="brow", bufs=2, name="bv_sb")
                    ones = sb.tile([1, P], F16, tag="ones", bufs=1, name="ones")
                    nc.sync.dma_start(out=bk_sb[:], in_=bkr[:])
                    nc.sync.dma_start(out=bv_sb[:], in_=bvr[:])
                    nc.vector.memset(ones[:], 1.0)
                wq_sb = [sb.tile([P, C], F16, tag="wt", bufs=40, name=f"wq{i}")
                         for i in range(KT)]
                w1_sb = [sb.tile([P, C], F16, tag="wt", bufs=40, name=f"w1{i}")
                         for i in range(KT)]
                w2_sb = [sb.tile([P, C], F16, tag="wt", bufs=40, name=f"w2{i}")
                         for i in range(KT)]
                for i in range(KT):
                    nc.sync.dma_start(out=xT_sb[i][:], in_=xT[i * P:(i + 1) * P, :])
                    nc.sync.dma_start(out=wq_sb[i][:], in_=wq[i * P:(i + 1) * P, :])
                for i in range(KT):
                    nc.sync.dma_start(out=w1_sb[i][:], in_=w1[i * P:(i + 1) * P, :])
                for i in range(KT):
                    nc.sync.dma_start(out=w2_sb[i][:], in_=w2[i * P:(i + 1) * P, :])

                # ---- phase 1: K/V (sequence-major) and kv partial ----
                # Every stationary serves both feature halves (paired matmuls
                # into two PSUM banks) so the PE loads each stationary once
                # per two matmuls.  kv pairs two heads per [128,128] k
                # stationary; the cross-head half of each product lands in
                # the unused partition half of its bank (bank A even heads at
                # partitions 0:64, bank B odd heads at 64:128, pair i at
                # free i*128).  kv matmuls are interleaved into the next
                # projection's stream via `extras` so their LDWEIGHTS hide.
                kv_ps_a = psp.tile([P, NB], F32, tag="acc", bufs=2, name="kv_ps_a")
                kv_ps_b = psp.tile([P, NB], F32, tag="acc", bufs=2, name="kv_ps_b")

                def proj_seq(w_sb, brow, st, nm, extras=None):
                    ssl = slice(st * P, (st + 1) * P)
                    t = sb.tile([P, C], F16, tag="kvs", bufs=5, name=nm)
                    pk0 = psp.tile([P, NB], F32, tag="mm", bufs=6, name="pk0")
                    pk1 = psp.tile([P, NB], F32, tag="mm", bufs=6, name="pk1")
                    ex = list(extras) if extras else []
                    for kt in range(KT):
                        for nb, pk in ((0, pk0), (1, pk1)):
                            nc.tensor.matmul(
                                pk[:],
                                lhsT=dT_sb[kt][:, ssl],
                                rhs=w_sb[kt][:, nb * NB:(nb + 1) * NB],
                                start=(kt == 0),
                                stop=(kt == KT - 1 and not with_bkv),
                            )
                        if ex:
                            ex.pop(0)()
                    if with_bkv:
                        nc.tensor.matmul(pk0[:], lhsT=ones[:],
                                         rhs=brow[:, 0:NB],
                                         start=False, stop=True)
                        nc.tensor.matmul(pk1[:], lhsT=ones[:],
                                         rhs=brow[:, NB:C],
                                         start=False, stop=True)
                    nc.scalar.copy(t[:, 0:NB], pk0[:])
                    nc.vector.tensor_copy(t[:, NB:C], pk1[:])
                    return t

                def proj_seq_f8(w8_sb_, st, nm, extras=None):
                    # DoubleRow: contraction 256 per pass, stationary is the
                    # fp8 data slice [128,2,128], moving the fp8 weight
                    # [128,2,512].  Evictions undo the 2^15 pre-scale.
                    t = sb.tile([P, C], F16, tag="kvs", bufs=5, name=nm)
                    pk0 = psp.tile([P, NB], F32, tag="mm", bufs=6, name="pk0")
                    pk1 = psp.tile([P, NB], F32, tag="mm", bufs=6, name="pk1")
                    ex = list(extras) if extras else []
                    for tt in range(DT8):
                        for nb, pk in ((0, pk0), (1, pk1)):
                            nc.tensor.matmul(
                                pk[:],
                                lhsT=dT8_sb[tt][:, :, st * P:(st + 1) * P],
                                rhs=w8_sb_[tt][:, :, nb * NB:(nb + 1) * NB],
                                start=(tt == 0),
                                stop=(tt == DT8 - 1),
                                perf_mode=DRM,
                            )
                        while ex:
                            ex.pop(0)()
                    nc.scalar.activation(t[:, 0:NB], pk0[:], AF.Copy, scale=SINV)
                    nc.vector.tensor_scalar_mul(t[:, NB:C], pk1[:], SINV)
                    return t

                def kv_thunks(k_t, v_t, st):
                    # pair i: full [128,128] product of the pair's k and v
                    # slices.  Quadrants [0:64,0:64] and [64:128,64:128] are
                    # the two heads' kv; the off-diagonal quadrants are
                    # cross-head junk, zeroed after the AllReduce.
                    def mk(i):
                        def go():
                            tgt = kv_ps_a if i < 4 else kv_ps_b
                            fo = (i % 4) * 128
                            nc.tensor.matmul(
                                tgt[:, fo:fo + 128],
                                lhsT=k_t[:, i * 128:(i + 1) * 128],
                                rhs=v_t[:, i * 128:(i + 1) * 128],
                                start=(st == 0 and i % 4 == 0),
                                stop=(st == MT - 1 and i % 4 == 3),
                            )
                        return go
                    return [mk(i) for i in range(8)]

                def PROJ_K(st, extras=None):
                    if f8:
                        return proj_seq_f8(wk8_sb, st, "k_t", extras)
                    return proj_seq(wk_sb, bk_sb if with_bkv else None, st,
                                    "k_t", extras)

                def PROJ_V(st, extras=None):
                    if f8 == "kv":
                        return proj_seq_f8(wv8_sb, st, "v_t", extras)
                    return proj_seq(wv_sb, bv_sb if with_bkv else None, st,
                                    "v_t", extras)

                pend = None
                for st in range(MT):
                    k_t = PROJ_K(st)
                    v_t = PROJ_V(st, extras=kv_thunks(*pend) if pend else None)
                    pend = (k_t, v_t, st)
                kv7 = kv_thunks(*pend)

                # ---- phase 2: AllReduce kv across the pair ----
                # kv_sb [128, 1024]: pair i at cols i*128 (junk rides along;
                # junk+junk is still junk and gets zeroed after the reduce).
                kv_sb = sb.tile([P, C], F16, tag="kvio", bufs=2, name="kv_sb")
                kv_in = drp.tile([P, C], F16, tag="dri", bufs=1, name="kv_in")
                kv_out = drp.tile([P, C], F16, tag="dro", bufs=1, name="kv_out")
                kv_red = sb.tile([P, C], F16, tag="kvr", bufs=2, name="kv_red")

                def kv_exchange():
                    nc.vector.tensor_copy(kv_sb[:, 0:NB], kv_ps_a[:])
                    nc.vector.tensor_copy(kv_sb[:, NB:C], kv_ps_b[:])
                    nc.gpsimd.dma_start(out=kv_in[:], in_=kv_sb[:])
                    if n_cores == 1 or not use_cc:
                        # single-core analysis build: stand in for the AllReduce
                        nc.gpsimd.dma_start(out=kv_out[:], in_=kv_in[:])
                    else:
                        nc.gpsimd.collective_compute(
                            "AllReduce", ALU.add, replica_groups=groups,
                            ins=[kv_in.opt()], outs=[kv_out.opt()],
                        )
                    nc.gpsimd.dma_start(out=kv_red[:], in_=kv_out[:])
                    # zero the cross-head quadrants so attention can contract
                    # over the full 128 partitions in one matmul per pair
                    for i in range(8):
                        co = i * 128
                        nc.vector.memset(kv_red[64:128, co:co + 64], 0.0)
                        nc.vector.memset(kv_red[0:64, co + 64:co + 128], 0.0)

                if phases <= 1:
                    for th in kv7:
                        th()
                    kv_exchange()
                    stg0 = sb.tile([P, NB], ODT, tag="stg", bufs=3, name="stg0")
                    nc.vector.tensor_copy(stg0[:], kv_red[:, 0:NB])
                    for m in range(MT):
                        for nb in range(NBLK):
                            nsl = slice(nb * NB, (nb + 1) * NB)
                            nc.gpsimd.dma_start(out=outT[m * P:(m + 1) * P, nsl],
                                                in_=stg0[:])
                    return

                # ---- phase 3: Q projection (all blocks), then attention ----
                # All 16 Q blocks precede the attention matmuls in PE order,
                # so the AllReduce has the full ~27us Q window to complete.
                # The last sequence block's kv matmuls ride in the first Q
                # blocks (from the 3rd stationary on, so the v_t eviction has
                # time to land).
                attn_sb = [sb.tile([P, SH], F16, tag="at", bufs=8, name=f"attn{m}")
                           for m in range(MT)]
                exchanged = []

                def q_extra():
                    if kv7:
                        kv7.pop(0)()
                    elif not exchanged:
                        exchanged.append(1)
                        kv_exchange()

                qpend = []
                nstat = 0
                for m in range(MT):
                    qp0 = psp.tile([P, NB], F32, tag="mm", bufs=6, name="qp0")
                    qp1 = psp.tile([P, NB], F32, tag="mm", bufs=6, name="qp1")
                    for kt in range(KT):
                        for nb, qp in ((0, qp0), (1, qp1)):
                            nc.tensor.matmul(
                                qp[:],
                                lhsT=wq_sb[kt][:, m * P:(m + 1) * P],
                                rhs=xT_sb[kt][:, nb * NB:(nb + 1) * NB],
                                start=(kt == 0), stop=(kt == KT - 1),
                            )
                        nstat += 1
                        if nstat >= 3:
                            q_extra()
                    for nb, qp in ((0, qp0), (1, qp1)):
                        qt = sb.tile([P, NB], F16, tag="qt", bufs=17, name="qt")
                        nc.scalar.activation(qt[:], qp[:], AF.Identity,
                                             bias=bq_sb[:, m:m + 1], scale=1.0)
                        qpend.append((m, nb, qt))
                while kv7:
                    kv7.pop(0)()
                if not exchanged:
                    kv_exchange()
                # nb-major attention order: the 8 nb=0 evictions complete
                # first, so MLP1's nb=0 sweep (which contracts over all 8
                # pairs at one sequence block) starts ~5us earlier.
                qpend.sort(key=lambda t: (t[1], t[0]))
                for m, nb, qt in qpend:
                    # qt partitions are [head 2m d | head 2m+1 d], matching
                    # the kv pair-block rows; with the junk quadrants zeroed
                    # one 128-contraction matmul yields both heads' attn in
                    # the right partition layout.
                    nsl = slice(nb * NB, (nb + 1) * NB)
                    ap_ = psp.tile([P, NB], F32, tag="mm", bufs=6, name="ap_")
                    nc.tensor.matmul(
                        ap_[:],
                        lhsT=kv_red[:, m * 128:(m + 1) * 128],
                        rhs=qt[:], start=True, stop=True)
                    nc.vector.tensor_copy(attn_sb[m][:, nsl], ap_[:])

                if phases <= 2:
                    for m in range(MT):
                        for nb in range(NBLK):
                            nsl = slice(nb * NB, (nb + 1) * NB)
                            nc.gpsimd.dma_start(out=outT[m * P:(m + 1) * P, nsl],
                                                in_=attn_sb[m][:, nsl])
                    return

                # ---- phase 5: MLP hidden with fused exact GELU ----
                h1_sb = [sb.tile([P, SH], F16, tag="h1", bufs=8, name=f"h1{m}")
                         for m in range(MT)]
                for nb in range(NBLK):
                    nsl = slice(nb * NB, (nb + 1) * NB)
                    for m in range(MT):
                        hp = psp.tile([P, NB], F32, tag="mm", bufs=6, name="hp")
                        for kt in range(KT):
                            nc.tensor.matmul(
                                hp[:],
                                lhsT=w1_sb[kt][:, m * P:(m + 1) * P],
                                rhs=attn_sb[kt][:, nsl],
                                start=(kt == 0), stop=(kt == KT - 1),
                            )
                        nc.scalar.activation(h1_sb[m][:, nsl], hp[:],
                                             AF.Gelu if gelu else AF.Identity,
                                             bias=b1_sb[:, m:m + 1], scale=1.0)

                if phases <= 3:
                    for m in range(MT):
                        for nb in range(NBLK):
                            nsl = slice(nb * NB, (nb + 1) * NB)
                            nc.gpsimd.dma_start(out=outT[m * P:(m + 1) * P, nsl],
                                                in_=h1_sb[m][:, nsl])
                    return

                # fold the attention output into the residual so the MLP2
                # eviction is a single DVE op per block
                for m in range(MT):
                    nc.vector.tensor_add(xT_sb[m][:], xT_sb[m][:],
                                         attn_sb[m][:])

                # ---- phase 6: MLP out + residuals; stores on Pool queue ----
                for nb in range(NBLK):
                    nsl = slice(nb * NB, (nb + 1) * NB)
                    for m in range(MT):
                        op = psp.tile([P, NB], F32, tag="mm", bufs=6, name="op")
                        for kt in range(KT):
                            nc.tensor.matmul(
                                op[:],
                                lhsT=w2_sb[kt][:, m * P:(m + 1) * P],
                                rhs=h1_sb[kt][:, nsl],
                                start=(kt == 0), stop=(kt == KT - 1),
                            )
                        stg = sb.tile([P, NB], ODT, tag="stg", bufs=3, name="stg")
                        # stg = (op + b2) + (x + attn)   [attn pre-folded into xT]
                        nc.vector.scalar_tensor_tensor(
                            stg[:], op[:], b2_sb[:, m:m + 1],
                            xT_sb[m][:, nsl], op0=ALU.add, op1=ALU.add)
                        nc.gpsimd.dma_start(
                            out=outT[m * P:(m + 1) * P, nsl], in_=stg[:])

            # Straight-line unroll for timing runs (collectives cannot sit
            # inside a hardware For_i loop on this execution path).
            for _ in range(loop_r):
                body()

    nc.compile()
    return nc


def _get_program(with_bkv: bool, loop_r: int = 1, use_cc: bool = True,
                 phases: int = 4, f8: str = F8):
    key = (with_bkv, loop_r, use_cc, phases, f8)
    if key not in _CACHE:
        _CACHE[key] = _build(with_bkv, loop_r, use_cc=use_cc, phases=phases,
                             f8=f8)
    return _CACHE[key]


def _pack3(a8):
    """[C, w] (contraction-major) -> [DT8*128, 2, w] DoubleRow interleave."""
    cw = a8.shape[1]
    return np.ascontiguousarray(
        a8.reshape(DT8, 2, P, cw).transpose(0, 2, 1, 3).reshape(DT8 * P, 2, cw)
    )


def _pack_inputs(x, data, Wq, bq, Wk, bk, Wv, bv, W1, b1, W2, b2, with_bkv,
                 f8: str = F8):
    import ml_dtypes
    f32 = np.float32
    f16 = {"f16": np.float16, "bf16": ml_dtypes.bfloat16,
           "f32r": np.float32}[GDT]
    f8dt = ml_dtypes.float8_e4m3
    wq_s = np.ascontiguousarray(np.asarray(Wq, f32) * f32(SCALE), dtype=f32).astype(f16)
    w1_c = np.asarray(W1, f32).astype(f16)
    w2_c = np.asarray(W2, f32).astype(f16)
    if f8:
        wkf = np.asarray(Wk, f32) * f32(SW8)
        assert np.abs(wkf).max() < 240.0, "Wk fp8 overflow"
        wk8_c = _pack3(wkf.astype(f8dt))
    else:
        wk_c = np.asarray(Wk, f32).astype(f16)
    if f8 == "kv":
        wvf = np.asarray(Wv, f32) * f32(SW8)
        assert np.abs(wvf).max() < 240.0, "Wv fp8 overflow"
        wv8_c = _pack3(wvf.astype(f8dt))
    else:
        wv_c = np.asarray(Wv, f32).astype(f16)
    bqt = np.ascontiguousarray((np.asarray(bq, f32) * f32(SCALE)).reshape(MT, P).T)
    b1t = np.ascontiguousarray(np.asarray(b1, f32).reshape(MT, P).T)
    b2t = np.ascontiguousarray(np.asarray(b2, f32).reshape(MT, P).T)
    in_maps = []
    for c in range(8):
        b_, j = divmod(c, 2)
        dTf = np.ascontiguousarray(np.asarray(data, f32)[b_, j * SH:(j + 1) * SH, :].T)
        m = {
            "xT": np.ascontiguousarray(np.asarray(x, f32)[b_, j * SH:(j + 1) * SH, :].T).astype(f16),
            "wq": wq_s, "w1": w1_c, "w2": w2_c,
            "bqt": bqt, "b1t": b1t, "b2t": b2t,
        }
        if f8:
            d8 = dTf * f32(SD8)
            assert np.abs(d8).max() < 240.0, "data fp8 overflow"
            m["dT8"] = _pack3(d8.astype(f8dt))
            m["wk8"] = wk8_c
        else:
            m["wk"] = wk_c
        if f8 == "kv":
            m["wv8"] = wv8_c
        else:
            m["wv"] = wv_c
        if f8 != "kv":
            m["dT"] = dTf.astype(f16)
        if with_bkv:
            m["bkr"] = np.asarray(bk, f32).reshape(1, C).astype(f16)
            m["bvr"] = np.asarray(bv, f32).reshape(1, C).astype(f16)
        in_maps.append(m)
    return in_maps


def run_on_hw(inputs, loop_r: int = 1, trace: bool = False):
    """Run the SPMD program; returns BassKernelResults."""
    from concourse.bass_utils import run_bass_kernel_spmd

    with_bkv = bool(
        np.any(np.asarray(inputs["bk"])) or np.any(np.asarray(inputs["bv"]))
    )
    f8 = "" if with_bkv else F8
    nc = _get_program(with_bkv, loop_r, f8=f8)
    in_maps = _pack_inputs(
        inputs["x"], inputs["data"], inputs["Wq"], inputs["bq"], inputs["Wk"],
        inputs["bk"], inputs["Wv"], inputs["bv"], inputs["W1"], inputs["b1"],
        inputs["W2"], inputs["b2"], with_bkv, f8=f8,
    )
    res = run_bass_kernel_spmd(nc, in_maps, list(range(8)), trace=trace)
    return res


def kernel(**inputs) -> np.ndarray:
    res = run_on_hw(inputs, loop_r=1)
    out = np.empty((B, S, C), dtype=np.float32)
    for c in range(8):
        b_, j = divmod(c, 2)
        out[b_, j * SH:(j + 1) * SH, :] = res.results[c]["outT"].T.astype(np.float32)
    return out


# revision 5
# speedup vs baseline: 1.1628x; 1.0110x over previous
"""Trainium2 Bass kernel for nn_Attention_56736517980393.

Reference computation (B=4, S=2048, C=1024, H=16 heads, D=64, MLP hidden 1024):
    q = (x @ Wq + bq) * D**-0.5          per-head [B,H,S,D]
    k = data @ Wk + bk ; v = data @ Wv + bv
    kv[b,h] = k^T @ v                     [D,D]   (no softmax -> associative form)
    attn = q @ kv                         [B,S,C]
    out = x + attn + gelu(attn @ W1 + b1) @ W2 + b2

Sharding: 8 cores = (batch b in 0..3) x (sequence half j in 0..1).
Each core computes K/V projections for its (b, j) sequence half, a partial
kv (reduced over its half), AllReduces kv with its pair core, then computes
Q / attn / MLP / residual for its half.  Activations are kept feature-major
(transposed, [C, S]) on chip so that biases are per-partition and no on-chip
transposes are needed; K and V are produced sequence-major for the kv matmul
by using the activation tile as the stationary matmul operand.

Precision: the K and V projections run in fp8e4 (TRN FP8_EXP4, max 240)
with perf_mode=DoubleRow -- 256-deep contraction per pass, ~1.8x the bf16
column rate -- with power-of-2 pre-scales (data x32, weights x1024) undone
exactly at the PSUM eviction (x 2^-15).  Everything else stays bf16.
End-to-end max-rel error vs the fp32 reference is ~1.9e-2 against the
harness budget of 2e-2 (deterministic inputs; fp8 numerics verified
bit-faithful to the ml_dtypes emulation).  Set F8=k for K-only (~1.2e-2)
or F8= (empty) for the all-bf16 variant (~5.3e-3).

The kv matmuls (128-col, one stationary each) are LDWEIGHTS-bound when
emitted as a block (107ns load vs 53ns stream); they are interleaved into
the following projection's matmul stream so each load hides under a
512-col matmul.

DMA queues: bulk loads ride the SP queue in exact consumption order;
the kv AllReduce staging and output stores ride the GpSimd SWDGE (the
Pool sequencer is otherwise idle).  Phase-1 PSUM evictions split across
Act and DVE.
"""

import os
import numpy as np

GDT = os.environ.get("GDT", "f16")
F8 = os.environ.get("F8", "kv")          # "kv" | "k" | ""

B, S, C, H, D = 4, 2048, 1024, 16, 64
SH = S // 2          # sequence rows per core
SCALE = D ** -0.5
P = 128              # SBUF partitions
NB = 512             # matmul moving free-dim block (one PSUM bank of fp32)
KT = C // P          # 8 contraction tiles
MT = C // P          # 8 output-feature tiles
NBLK = SH // NB      # 2 sequence blocks
DT8 = C // 256       # 4 DoubleRow contraction tiles
SD8 = 32.0           # fp8 pre-scale on data
SW8 = 1024.0         # fp8 pre-scale on Wk/Wv
SINV = 1.0 / (SD8 * SW8)

_CACHE = {}


def _build(with_bkv: bool, loop_r: int = 1, n_cores: int = 8, use_cc: bool = True,
           phases: int = 4, f8: str = F8, gelu: bool = True):
    import concourse.bacc as bacc
    import concourse.mybir as mybir
    from concourse.tile import TileContext

    assert not (with_bkv and f8), "fp8 path keeps biases unsupported; use F8="
    F32 = mybir.dt.float32
    F16 = {"f16": mybir.dt.float16, "bf16": mybir.dt.bfloat16,
           "f32r": mybir.dt.float32r}[GDT]
    F8E4 = mybir.dt.float8e4
    DRM = mybir.MatmulPerfMode.DoubleRow
    ODT = mybir.dt.float16
    AF = mybir.ActivationFunctionType
    ALU = mybir.AluOpType

    nc = bacc.Bacc(
        trn_type="TRN2", target_bir_lowering=False, debug=False, num_devices=n_cores
    )

    xT = nc.dram_tensor("xT", [C, SH], F16, kind="ExternalInput").ap()
    if f8 != "kv":
        dT = nc.dram_tensor("dT", [C, SH], F16, kind="ExternalInput").ap()
    if f8:
        dT8 = nc.dram_tensor("dT8", [DT8 * P, 2, SH], F8E4,
                             kind="ExternalInput").ap()
        wk8 = nc.dram_tensor("wk8", [DT8 * P, 2, C], F8E4,
                             kind="ExternalInput").ap()
    else:
        wk = nc.dram_tensor("wk", [C, C], F16, kind="ExternalInput").ap()
    if f8 == "kv":
        wv8 = nc.dram_tensor("wv8", [DT8 * P, 2, C], F8E4,
                             kind="ExternalInput").ap()
    else:
        wv = nc.dram_tensor("wv", [C, C], F16, kind="ExternalInput").ap()
    wq = nc.dram_tensor("wq", [C, C], F16, kind="ExternalInput").ap()
    w1 = nc.dram_tensor("w1", [C, C], F16, kind="ExternalInput").ap()
    w2 = nc.dram_tensor("w2", [C, C], F16, kind="ExternalInput").ap()
    # feature-major biases: [128, 8] so that column m is the per-partition
    # bias for feature tile m
    bqt = nc.dram_tensor("bqt", [P, MT], F32, kind="ExternalInput").ap()
    b1t = nc.dram_tensor("b1t", [P, MT], F32, kind="ExternalInput").ap()
    b2t = nc.dram_tensor("b2t", [P, MT], F32, kind="ExternalInput").ap()
    if with_bkv:
        bkr = nc.dram_tensor("bkr", [1, C], F16, kind="ExternalInput").ap()
        bvr = nc.dram_tensor("bvr", [1, C], F16, kind="ExternalInput").ap()
    outT = nc.dram_tensor("outT", [C, SH], ODT, kind="ExternalOutput").ap()

    groups = [[i, i + 1] for i in range(0, n_cores, 2)]

    with TileContext(nc) as tc:
        with tc.tile_pool(name="sb", bufs=1) as sb, \
             tc.tile_pool(name="ps", bufs=1, space="PSUM") as psp, \
             tc.tile_pool(name="dr", bufs=1, space="DRAM") as drp:

            # One-time act-table preload: the gelu_and_others set also holds
            # Identity and Copy, so every later Act op is served without a
            # 1.3us table swap.
            if gelu:
                warm = sb.tile([1, 8], F32, tag="warm", bufs=1, name="warm")
                nc.vector.memset(warm[:], 0.0)
                nc.scalar.activation(warm[:], warm[:], AF.Gelu, scale=1.0)

            def body(it=None):
                # ---- SBUF tiles ----
                if f8:
                    dT8_sb = [sb.tile([P, 2, SH], F8E4, tag="d8", bufs=DT8,
                                      name=f"dT8{t}") for t in range(DT8)]
                    wk8_sb = [sb.tile([P, 2, C], F8E4, tag="w8",
                                      bufs=(2 * DT8 if f8 == "kv" else DT8),
                                      name=f"wk8{t}") for t in range(DT8)]
                if f8 == "kv":
                    wv8_sb = [sb.tile([P, 2, C], F8E4, tag="w8", bufs=2 * DT8,
                                      name=f"wv8{t}") for t in range(DT8)]
                else:
                    wv_sb = [sb.tile([P, C], F16, tag="wt", bufs=40,
                                     name=f"wv{i}") for i in range(KT)]
                    dT_sb = [sb.tile([P, SH], F16, tag="dh", bufs=8,
                                     name=f"dT{i}") for i in range(KT)]
                if not f8:
                    wk_sb = [sb.tile([P, C], F16, tag="wt", bufs=40,
                                     name=f"wk{i}") for i in range(KT)]
                xT_sb = [sb.tile([P, SH], F16, tag="xa", bufs=8, name=f"xT{i}")
                         for i in range(KT)]
                # ---- SP load queue: strict phase-1 consumption order, then
                # next-phase prefetch.  Everything is resident well before use.
                if f8 == "kv":
                    for t in range(DT8):
                        nc.sync.dma_start(out=dT8_sb[t][:, :, 0:P],
                                          in_=dT8[t * P:(t + 1) * P, :, 0:P])
                        nc.sync.dma_start(out=wk8_sb[t][:], in_=wk8[t * P:(t + 1) * P, :, :])
                    for t in range(DT8):
                        nc.sync.dma_start(out=dT8_sb[t][:, :, P:SH],
                                          in_=dT8[t * P:(t + 1) * P, :, P:SH])
                    for t in range(DT8):
                        nc.sync.dma_start(out=wv8_sb[t][:], in_=wv8[t * P:(t + 1) * P, :, :])
                elif f8 == "k":
                    for t in range(DT8):
                        nc.sync.dma_start(out=dT8_sb[t][:, :, 0:P],
                                          in_=dT8[t * P:(t + 1) * P, :, 0:P])
                        nc.sync.dma_start(out=wk8_sb[t][:], in_=wk8[t * P:(t + 1) * P, :, :])
                    for i in range(KT):
                        nc.sync.dma_start(out=dT_sb[i][:, 0:P], in_=dT[i * P:(i + 1) * P, 0:P])
                    for t in range(DT8):
                        nc.sync.dma_start(out=dT8_sb[t][:, :, P:SH],
                                          in_=dT8[t * P:(t + 1) * P, :, P:SH])
                    for i in range(KT):
                        nc.sync.dma_start(out=dT_sb[i][:, P:SH], in_=dT[i * P:(i + 1) * P, P:SH])
                    for i in range(KT):
                        nc.sync.dma_start(out=wv_sb[i][:], in_=wv[i * P:(i + 1) * P, :])
                else:
                    for i in range(KT):
                        nc.sync.dma_start(out=dT_sb[i][:, 0:P], in_=dT[i * P:(i + 1) * P, 0:P])
                        nc.sync.dma_start(out=wk_sb[i][:], in_=wk[i * P:(i + 1) * P, :])
                    for i in range(KT):
                        nc.sync.dma_start(out=dT_sb[i][:, P:SH], in_=dT[i * P:(i + 1) * P, P:SH])
                    for i in range(KT):
                        nc.sync.dma_start(out=wv_sb[i][:], in_=wv[i * P:(i + 1) * P, :])
                bq_sb = sb.tile([P, MT], F32, tag="bias", bufs=3, name="bq_sb")
                b1_sb = sb.tile([P, MT], F32, tag="bias", bufs=3, name="b1_sb")
                b2_sb = sb.tile([P, MT], F32, tag="bias", bufs=3, name="b2_sb")
                nc.sync.dma_start(out=bq_sb[:], in_=bqt[:])
                nc.sync.dma_start(out=b1_sb[:], in_=b1t[:])
                nc.sync.dma_start(out=b2_sb[:], in_=b2t[:])
                if with_bkv:
                    bk_sb = sb.tile([1, C], F16, tag="brow", bufs=2, name="bk_sb")
                    bv_sb = sb.tile([1, C], F16, tag="brow", bufs=2, name="bv_sb")
                    ones = sb.tile([1, P], F16, tag="ones", bufs=1, name="ones")
                    nc.sync.dma_start(out=bk_sb[:], in_=bkr[:])
                    nc.sync.dma_start(out=bv_sb[:], in_=bvr[:])
                    nc.vector.memset(ones[:], 1.0)
                wq_sb = [sb.tile([P, C], F16, tag="wt", bufs=40, name=f"wq{i}")
                         for i in range(KT)]
                w1_sb = [sb.tile([P, C], F16, tag="wt", bufs=40, name=f"w1{i}")
                         for i in range(KT)]
                w2_sb = [sb.tile([P, C], F16, tag="wt", bufs=40, name=f"w2{i}")
                         for i in range(KT)]
                for i in range(KT):
                    nc.sync.dma_start(out=xT_sb[i][:], in_=xT[i * P:(i + 1) * P, :])
                    nc.sync.dma_start(out=wq_sb[i][:], in_=wq[i * P:(i + 1) * P, :])
                for i in range(KT):
                    nc.sync.dma_start(out=w1_sb[i][:], in_=w1[i * P:(i + 1) * P, :])
                for i in range(KT):
                    nc.sync.dma_start(out=w2_sb[i][:], in_=w2[i * P:(i + 1) * P, :])

                # ---- phase 1: K/V (sequence-major) and kv partial ----
                # Every stationary serves both feature halves (paired matmuls
                # into two PSUM banks) so the PE loads each stationary once
                # per two matmuls.  kv pairs two heads per [128,128] k
                # stationary; the cross-head half of each product lands in
                # the unused partition half of its bank (bank A even heads at
                # partitions 0:64, bank B odd heads at 64:128, pair i at
                # free i*128).  kv matmuls are interleaved into the next
                # projection's stream via `extras` so their LDWEIGHTS hide.
                kv_ps_a = psp.tile([P, NB], F32, tag="acc", bufs=2, name="kv_ps_a")
                kv_ps_b = psp.tile([P, NB], F32, tag="acc", bufs=2, name="kv_ps_b")

                def proj_seq(w_sb, brow, st, nm, extras=None):
                    ssl = slice(st * P, (st + 1) * P)
                    t = sb.tile([P, C], F16, tag="kvs", bufs=5, name=nm)
                    pk0 = psp.tile([P, NB], F32, tag="mm", bufs=6, name="pk0")
                    pk1 = psp.tile([P, NB], F32, tag="mm", bufs=6, name="pk1")
                    ex = list(extras) if extras else []
                    for kt in range(KT):
                        for nb, pk in ((0, pk0), (1, pk1)):
                            nc.tensor.matmul(
                                pk[:],
                                lhsT=dT_sb[kt][:, ssl],
                                rhs=w_sb[kt][:, nb * NB:(nb + 1) * NB],
                                start=(kt == 0),
                                stop=(kt == KT - 1 and not with_bkv),
                            )
                            if ex:
                                ex.pop(0)()
                    if with_bkv:
                        nc.tensor.matmul(pk0[:], lhsT=ones[:],
                                         rhs=brow[:, 0:NB],
                                         start=False, stop=True)
                        nc.tensor.matmul(pk1[:], lhsT=ones[:],
                                         rhs=brow[:, NB:C],
                                         start=False, stop=True)
                    nc.scalar.copy(t[:, 0:NB], pk0[:])
                    nc.vector.tensor_copy(t[:, NB:C], pk1[:])
                    return t

                def proj_seq_f8(w8_sb_, st, nm, extras=None):
                    # DoubleRow: contraction 256 per pass, stationary is the
                    # fp8 data slice [128,2,128], moving the fp8 weight
                    # [128,2,512].  Evictions undo the 2^15 pre-scale.
                    t = sb.tile([P, C], F16, tag="kvs", bufs=5, name=nm)
                    pk0 = psp.tile([P, NB], F32, tag="mm", bufs=6, name="pk0")
                    pk1 = psp.tile([P, NB], F32, tag="mm", bufs=6, name="pk1")
                    ex = list(extras) if extras else []
                    for tt in range(DT8):
                        for nb, pk in ((0, pk0), (1, pk1)):
                            nc.tensor.matmul(
                                pk[:],
                                lhsT=dT8_sb[tt][:, :, st * P:(st + 1) * P],
                                rhs=w8_sb_[tt][:, :, nb * NB:(nb + 1) * NB],
                                start=(tt == 0),
                                stop=(tt == DT8 - 1),
                                perf_mode=DRM,
                            )
                            # one kv matmul per DR matmul: its 107ns
                            # LDWEIGHTS hides under the 241ns DR stream
                            if ex:
                                ex.pop(0)()
                    nc.scalar.activation(t[:, 0:NB], pk0[:], AF.Copy, scale=SINV)
                    nc.vector.tensor_scalar_mul(t[:, NB:C], pk1[:], SINV)
                    return t

                def kv_thunks(k_t, v_t, st):
                    # pair i: full [128,128] product of the pair's k and v
                    # slices.  Quadrants [0:64,0:64] and [64:128,64:128] are
                    # the two heads' kv; the off-diagonal quadrants are
                    # cross-head junk, zeroed after the AllReduce.
                    def mk(i):
                        def go():
                            tgt = kv_ps_a if i < 4 else kv_ps_b
                            fo = (i % 4) * 128
                            nc.tensor.matmul(
                                tgt[:, fo:fo + 128],
                                lhsT=k_t[:, i * 128:(i + 1) * 128],
                                rhs=v_t[:, i * 128:(i + 1) * 128],
                                start=(st == 0 and i % 4 == 0),
                                stop=(st == MT - 1 and i % 4 == 3),
                            )
                        return go
                    return [mk(i) for i in range(8)]

                def PROJ_K(st, extras=None):
                    if f8:
                        return proj_seq_f8(wk8_sb, st, "k_t", extras)
                    return proj_seq(wk_sb, bk_sb if with_bkv else None, st,
                                    "k_t", extras)

                def PROJ_V(st, extras=None):
                    if f8 == "kv":
                        return proj_seq_f8(wv8_sb, st, "v_t", extras)
                    return proj_seq(wv_sb, bv_sb if with_bkv else None, st,
                                    "v_t", extras)

                pend = None
                for st in range(MT):
                    k_t = PROJ_K(st)
                    v_t = PROJ_V(st, extras=kv_thunks(*pend) if pend else None)
                    pend = (k_t, v_t, st)
                kv7 = kv_thunks(*pend)

                # ---- phase 2: AllReduce kv across the pair ----
                # kv_sb [128, 1024]: pair i at cols i*128 (junk rides along;
                # junk+junk is still junk and gets zeroed after the reduce).
                kv_sb = sb.tile([P, C], F16, tag="kvio", bufs=2, name="kv_sb")
                kv_in = drp.tile([P, C], F16, tag="dri", bufs=1, name="kv_in")
                kv_out = drp.tile([P, C], F16, tag="dro", bufs=1, name="kv_out")
                kv_red = sb.tile([P, C], F16, tag="kvr", bufs=2, name="kv_red")

                def kv_exchange():
                    nc.vector.tensor_copy(kv_sb[:, 0:NB], kv_ps_a[:])
                    nc.vector.tensor_copy(kv_sb[:, NB:C], kv_ps_b[:])
                    nc.gpsimd.dma_start(out=kv_in[:], in_=kv_sb[:])
                    if n_cores == 1 or not use_cc:
                        # single-core analysis build: stand in for the AllReduce
                        nc.gpsimd.dma_start(out=kv_out[:], in_=kv_in[:])
                    else:
                        nc.gpsimd.collective_compute(
                            "AllReduce", ALU.add, replica_groups=groups,
                            ins=[kv_in.opt()], outs=[kv_out.opt()],
                        )
                    nc.gpsimd.dma_start(out=kv_red[:], in_=kv_out[:])
                    # zero the cross-head quadrants so attention can contract
                    # over the full 128 partitions in one matmul per pair
                    for i in range(8):
                        co = i * 128
                        nc.vector.memset(kv_red[64:128, co:co + 64], 0.0)
                        nc.vector.memset(kv_red[0:64, co + 64:co + 128], 0.0)

                if phases <= 1:
                    for th in kv7:
                        th()
                    kv_exchange()
                    stg0 = sb.tile([P, NB], ODT, tag="stg", bufs=3, name="stg0")
                    nc.vector.tensor_copy(stg0[:], kv_red[:, 0:NB])
                    for m in range(MT):
                        for nb in range(NBLK):
                            nsl = slice(nb * NB, (nb + 1) * NB)
                            nc.gpsimd.dma_start(out=outT[m * P:(m + 1) * P, nsl],
                                                in_=stg0[:])
                    return

                # ---- phase 3: Q projection (all blocks), then attention ----
                # All 16 Q blocks precede the attention matmuls in PE order,
                # so the AllReduce has the full ~27us Q window to complete.
                # The last sequence block's kv matmuls ride in the first Q
                # blocks (from the 3rd stationary on, so the v_t eviction has
                # time to land).
                attn_sb = [sb.tile([P, SH], F16, tag="at", bufs=8, name=f"attn{m}")
                           for m in range(MT)]
                exchanged = []

                def q_extra():
                    if kv7:
                        kv7.pop(0)()
                    elif not exchanged:
                        exchanged.append(1)
                        kv_exchange()

                qpend = []
                nstat = 0
                for m in range(MT):
                    qp0 = psp.tile([P, NB], F32, tag="mm", bufs=6, name="qp0")
                    qp1 = psp.tile([P, NB], F32, tag="mm", bufs=6, name="qp1")
                    for kt in range(KT):
                        for nb, qp in ((0, qp0), (1, qp1)):
                            nc.tensor.matmul(
                                qp[:],
                                lhsT=wq_sb[kt][:, m * P:(m + 1) * P],
                                rhs=xT_sb[kt][:, nb * NB:(nb + 1) * NB],
                                start=(kt == 0), stop=(kt == KT - 1),
                            )
                        nstat += 1
                        if nstat >= 3:
                            q_extra()
                    for nb, qp in ((0, qp0), (1, qp1)):
                        qt = sb.tile([P, NB], F16, tag="qt", bufs=17, name="qt")
                        nc.scalar.activation(qt[:], qp[:], AF.Identity,
                                             bias=bq_sb[:, m:m + 1], scale=1.0)
                        qpend.append((m, nb, qt))
                while kv7:
                    kv7.pop(0)()
                if not exchanged:
                    kv_exchange()
                # nb-major attention order: the 8 nb=0 evictions complete
                # first, so MLP1's nb=0 sweep (which contracts over all 8
                # pairs at one sequence block) starts ~5us earlier.
                qpend.sort(key=lambda t: (t[1], t[0]))
                for m, nb, qt in qpend:
                    # qt partitions are [head 2m d | head 2m+1 d], matching
                    # the kv pair-block rows; with the junk quadrants zeroed
                    # one 128-contraction matmul yields both heads' attn in
                    # the right partition layout.
                    nsl = slice(nb * NB, (nb + 1) * NB)
                    ap_ = psp.tile([P, NB], F32, tag="mm", bufs=6, name="ap_")
                    nc.tensor.matmul(
                        ap_[:],
                        lhsT=kv_red[:, m * 128:(m + 1) * 128],
                        rhs=qt[:], start=True, stop=True)
                    nc.vector.tensor_copy(attn_sb[m][:, nsl], ap_[:])

                if phases <= 2:
                    for m in range(MT):
                        for nb in range(NBLK):
                            nsl = slice(nb * NB, (nb + 1) * NB)
                            nc.gpsimd.dma_start(out=outT[m * P:(m + 1) * P, nsl],
                                                in_=attn_sb[m][:, nsl])
                    return

                # ---- phase 5: MLP hidden with fused exact GELU ----
                h1_sb = [sb.tile([P, SH], F16, tag="h1", bufs=8, name=f"h1{m}")
                         for m in range(MT)]
                for nb in range(NBLK):
                    nsl = slice(nb * NB, (nb + 1) * NB)
                    for m in range(MT):
                        hp = psp.tile([P, NB], F32, tag="mm", bufs=6, name="hp")
                        for kt in range(KT):
                            nc.tensor.matmul(
                                hp[:],
                                lhsT=w1_sb[kt][:, m * P:(m + 1) * P],
                                rhs=attn_sb[kt][:, nsl],
                                start=(kt == 0), stop=(kt == KT - 1),
                            )
                        nc.scalar.activation(h1_sb[m][:, nsl], hp[:],
                                             AF.Gelu if gelu else AF.Identity,
                                             bias=b1_sb[:, m:m + 1], scale=1.0)

                if phases <= 3:
                    for m in range(MT):
                        for nb in range(NBLK):
                            nsl = slice(nb * NB, (nb + 1) * NB)
                            nc.gpsimd.dma_start(out=outT[m * P:(m + 1) * P, nsl],
                                                in_=h1_sb[m][:, nsl])
                    return

                # fold the attention output into the residual so the MLP2
                # eviction is a single DVE op per block
                for m in range(MT):
                    nc.vector.tensor_add(xT_sb[m][:], xT_sb[m][:],
                                         attn_sb[m][:])

                # ---- phase 6: MLP out + residuals; stores on Pool queue ----
                for nb in range(NBLK):
                    nsl = slice(nb * NB, (nb + 1) * NB)
                    for m in range(MT):
                        op = psp.tile([P, NB], F32, tag="mm", bufs=6, name="op")
                        for kt in range(KT):
                            nc.tensor.matmul(
                                op[:],
                                lhsT=w2_sb[kt][:, m * P:(m + 1) * P],
                                rhs=h1_sb[kt][:, nsl],
                                start=(kt == 0), stop=(kt == KT - 1),
                            )
                        stg = sb.tile([P, NB], ODT, tag="stg", bufs=3, name="stg")
                        # stg = (op + b2) + (x + attn)   [attn pre-folded into xT]
                        nc.vector.scalar_tensor_tensor(
                            stg[:], op[:], b2_sb[:, m:m + 1],
                            xT_sb[m][:, nsl], op0=ALU.add, op1=ALU.add)
                        nc.gpsimd.dma_start(
                            out=outT[m * P:(m + 1) * P, nsl], in_=stg[:])

            # Straight-line unroll for timing runs (collectives cannot sit
            # inside a hardware For_i loop on this execution path).
            for _ in range(loop_r):
                body()

    nc.compile()
    return nc


def _get_program(with_bkv: bool, loop_r: int = 1, use_cc: bool = True,
                 phases: int = 4, f8: str = F8):
    key = (with_bkv, loop_r, use_cc, phases, f8)
    if key not in _CACHE:
        _CACHE[key] = _build(with_bkv, loop_r, use_cc=use_cc, phases=phases,
                             f8=f8)
    return _CACHE[key]


def _pack3(a8):
    """[C, w] (contraction-major) -> [DT8*128, 2, w] DoubleRow interleave."""
    cw = a8.shape[1]
    return np.ascontiguousarray(
        a8.reshape(DT8, 2, P, cw).transpose(0, 2, 1, 3).reshape(DT8 * P, 2, cw)
    )


def _pack_inputs(x, data, Wq, bq, Wk, bk, Wv, bv, W1, b1, W2, b2, with_bkv,
                 f8: str = F8):
    import ml_dtypes
    f32 = np.float32
    f16 = {"f16": np.float16, "bf16": ml_dtypes.bfloat16,
           "f32r": np.float32}[GDT]
    f8dt = ml_dtypes.float8_e4m3
    wq_s = np.ascontiguousarray(np.asarray(Wq, f32) * f32(SCALE), dtype=f32).astype(f16)
    w1_c = np.asarray(W1, f32).astype(f16)
    w2_c = np.asarray(W2, f32).astype(f16)
    if f8:
        wkf = np.asarray(Wk, f32) * f32(SW8)
        assert np.abs(wkf).max() < 240.0, "Wk fp8 overflow"
        wk8_c = _pack3(wkf.astype(f8dt))
    else:
        wk_c = np.asarray(Wk, f32).astype(f16)
    if f8 == "kv":
        wvf = np.asarray(Wv, f32) * f32(SW8)
        assert np.abs(wvf).max() < 240.0, "Wv fp8 overflow"
        wv8_c = _pack3(wvf.astype(f8dt))
    else:
        wv_c = np.asarray(Wv, f32).astype(f16)
    bqt = np.ascontiguousarray((np.asarray(bq, f32) * f32(SCALE)).reshape(MT, P).T)
    b1t = np.ascontiguousarray(np.asarray(b1, f32).reshape(MT, P).T)
    b2t = np.ascontiguousarray(np.asarray(b2, f32).reshape(MT, P).T)
    in_maps = []
    for c in range(8):
        b_, j = divmod(c, 2)
        dTf = np.ascontiguousarray(np.asarray(data, f32)[b_, j * SH:(j + 1) * SH, :].T)
        m = {
            "xT": np.ascontiguousarray(np.asarray(x, f32)[b_, j * SH:(j + 1) * SH, :].T).astype(f16),
            "wq": wq_s, "w1": w1_c, "w2": w2_c,
            "bqt": bqt, "b1t": b1t, "b2t": b2t,
        }
        if f8:
            d8 = dTf * f32(SD8)
            assert np.abs(d8).max() < 240.0, "data fp8 overflow"
            m["dT8"] = _pack3(d8.astype(f8dt))
            m["wk8"] = wk8_c
        else:
            m["wk"] = wk_c
        if f8 == "kv":
            m["wv8"] = wv8_c
        else:
            m["wv"] = wv_c
        if f8 != "kv":
            m["dT"] = dTf.astype(f16)
        if with_bkv:
            m["bkr"] = np.asarray(bk, f32).reshape(1, C).astype(f16)
            m["bvr"] = np.asarray(bv, f32).reshape(1, C).astype(f16)
        in_maps.append(m)
    return in_maps


def run_on_hw(inputs, loop_r: int = 1, trace: bool = False):
    """Run the SPMD program; returns BassKernelResults."""
    from concourse.bass_utils import run_bass_kernel_spmd

    with_bkv = bool(
        np.any(np.asarray(inputs["bk"])) or np.any(np.asarray(inputs["bv"]))
    )
    f8 = "" if with_bkv else F8
    nc = _get_program(with_bkv, loop_r, f8=f8)
    in_maps = _pack_inputs(
        inputs["x"], inputs["data"], inputs["Wq"], inputs["bq"], inputs["Wk"],
        inputs["bk"], inputs["Wv"], inputs["bv"], inputs["W1"], inputs["b1"],
        inputs["W2"], inputs["b2"], with_bkv, f8=f8,
    )
    res = run_bass_kernel_spmd(nc, in_maps, list(range(8)), trace=trace)
    return res


def kernel(**inputs) -> np.ndarray:
    res = run_on_hw(inputs, loop_r=1)
    out = np.empty((B, S, C), dtype=np.float32)
    for c in range(8):
        b_, j = divmod(c, 2)
        out[b_, j * SH:(j + 1) * SH, :] = res.results[c]["outT"].T.astype(np.float32)
    return out


# revision 6
# speedup vs baseline: 1.3254x; 1.1399x over previous
"""Trainium2 Bass kernel for nn_Attention_56736517980393.

Reference computation (B=4, S=2048, C=1024, H=16 heads, D=64, MLP hidden 1024):
    q = (x @ Wq + bq) * D**-0.5          per-head [B,H,S,D]
    k = data @ Wk + bk ; v = data @ Wv + bv
    kv[b,h] = k^T @ v                     [D,D]   (no softmax -> associative form)
    attn = q @ kv                         [B,S,C]
    out = x + attn + gelu(attn @ W1 + b1) @ W2 + b2

Sharding: 8 cores = (batch b in 0..3) x (sequence half j in 0..1).
Each core computes K/V projections for its (b, j) sequence half, a partial
kv (reduced over its half), AllReduces kv with its pair core, then computes
Q / attn / MLP / residual for its half.  Activations are kept feature-major
(transposed, [C, S]) on chip so that biases are per-partition and no on-chip
transposes are needed; K and V are produced sequence-major for the kv matmul
by using the activation tile as the stationary matmul operand.

Precision: the K and V projections run in fp8e4 (TRN FP8_EXP4, max 240)
with perf_mode=DoubleRow -- 256-deep contraction per pass, ~1.8x the bf16
column rate -- with power-of-2 pre-scales (data x32, weights x1024) undone
exactly at the PSUM eviction (x 2^-15).  Everything else stays bf16.
End-to-end max-rel error vs the fp32 reference is ~1.9e-2 against the
harness budget of 2e-2 (deterministic inputs; fp8 numerics verified
bit-faithful to the ml_dtypes emulation).  Set F8=k for K-only (~1.2e-2)
or F8= (empty) for the all-bf16 variant (~5.3e-3).

The kv matmuls (128-col, one stationary each) are LDWEIGHTS-bound when
emitted as a block (107ns load vs 53ns stream); they are interleaved into
the following projection's matmul stream so each load hides under a
512-col matmul.

DMA queues: bulk loads ride the SP queue in exact consumption order;
the kv AllReduce staging and output stores ride the GpSimd SWDGE (the
Pool sequencer is otherwise idle).  Phase-1 PSUM evictions split across
Act and DVE.
"""

import os
import numpy as np

GDT = os.environ.get("GDT", "f16")
F8 = os.environ.get("F8", "kv")          # "kv" | "k" | ""

B, S, C, H, D = 4, 2048, 1024, 16, 64
SH = S // 2          # sequence rows per core
SCALE = D ** -0.5
P = 128              # SBUF partitions
NB = 512             # matmul moving free-dim block (one PSUM bank of fp32)
KT = C // P          # 8 contraction tiles
MT = C // P          # 8 output-feature tiles
NBLK = SH // NB      # 2 sequence blocks
DT8 = C // 256       # 4 DoubleRow contraction tiles
SD8 = 32.0           # fp8 pre-scale on data
SW8 = 1024.0         # fp8 pre-scale on Wk/Wv
SINV = 1.0 / (SD8 * SW8)

_CACHE = {}


def _build(with_bkv: bool, loop_r: int = 1, n_cores: int = 8, use_cc: bool = True,
           phases: int = 4, f8: str = F8, gelu: bool = True):
    import concourse.bacc as bacc
    import concourse.mybir as mybir
    from concourse.tile import TileContext

    assert not (with_bkv and f8), "fp8 path keeps biases unsupported; use F8="
    F32 = mybir.dt.float32
    F16 = {"f16": mybir.dt.float16, "bf16": mybir.dt.bfloat16,
           "f32r": mybir.dt.float32r}[GDT]
    F8E4 = mybir.dt.float8e4
    DRM = mybir.MatmulPerfMode.DoubleRow
    ODT = mybir.dt.float16
    AF = mybir.ActivationFunctionType
    ALU = mybir.AluOpType

    nc = bacc.Bacc(
        trn_type="TRN2", target_bir_lowering=False, debug=False, num_devices=n_cores
    )

    xT = nc.dram_tensor("xT", [C, SH], F16, kind="ExternalInput").ap()
    if f8 != "kv":
        dT = nc.dram_tensor("dT", [C, SH], F16, kind="ExternalInput").ap()
    if f8:
        dT8 = nc.dram_tensor("dT8", [DT8 * P, 2, SH], F8E4,
                             kind="ExternalInput").ap()
        wk8 = nc.dram_tensor("wk8", [DT8 * P, 2, C], F8E4,
                             kind="ExternalInput").ap()
    else:
        wk = nc.dram_tensor("wk", [C, C], F16, kind="ExternalInput").ap()
    if f8 == "kv":
        wv8 = nc.dram_tensor("wv8", [DT8 * P, 2, C], F8E4,
                             kind="ExternalInput").ap()
    else:
        wv = nc.dram_tensor("wv", [C, C], F16, kind="ExternalInput").ap()
    wq = nc.dram_tensor("wq", [C, C], F16, kind="ExternalInput").ap()
    w1 = nc.dram_tensor("w1", [C, C], F16, kind="ExternalInput").ap()
    w2 = nc.dram_tensor("w2", [C, C], F16, kind="ExternalInput").ap()
    # feature-major biases: [128, 8] so that column m is the per-partition
    # bias for feature tile m
    bqt = nc.dram_tensor("bqt", [P, MT], F32, kind="ExternalInput").ap()
    b1t = nc.dram_tensor("b1t", [P, MT], F32, kind="ExternalInput").ap()
    b2t = nc.dram_tensor("b2t", [P, MT], F32, kind="ExternalInput").ap()
    if with_bkv:
        bkr = nc.dram_tensor("bkr", [1, C], F16, kind="ExternalInput").ap()
        bvr = nc.dram_tensor("bvr", [1, C], F16, kind="ExternalInput").ap()
    outT = nc.dram_tensor("outT", [C, SH], ODT, kind="ExternalOutput").ap()

    groups = [[i, i + 1] for i in range(0, n_cores, 2)]

    with TileContext(nc) as tc:
        with tc.tile_pool(name="sb", bufs=1) as sb, \
             tc.tile_pool(name="ps", bufs=1, space="PSUM") as psp, \
             tc.tile_pool(name="dr", bufs=1, space="DRAM") as drp:

            # One-time act-table preload: the gelu_and_others set also holds
            # Identity and Copy, so every later Act op is served without a
            # 1.3us table swap.
            if gelu:
                warm = sb.tile([1, 8], F32, tag="warm", bufs=1, name="warm")
                nc.vector.memset(warm[:], 0.0)
                nc.scalar.activation(warm[:], warm[:], AF.Gelu, scale=1.0)

            def body(it=None):
                # ---- SBUF tiles ----
                if f8:
                    dT8_sb = [sb.tile([P, 2, SH], F8E4, tag="d8", bufs=DT8,
                                      name=f"dT8{t}") for t in range(DT8)]
                    wk8_sb = [sb.tile([P, 2, C], F8E4, tag="w8",
                                      bufs=(2 * DT8 if f8 == "kv" else DT8),
                                      name=f"wk8{t}") for t in range(DT8)]
                if f8 == "kv":
                    wv8_sb = [sb.tile([P, 2, C], F8E4, tag="w8", bufs=2 * DT8,
                                      name=f"wv8{t}") for t in range(DT8)]
                else:
                    wv_sb = [sb.tile([P, C], F16, tag="wt", bufs=40,
                                     name=f"wv{i}") for i in range(KT)]
                    dT_sb = [sb.tile([P, SH], F16, tag="dh", bufs=8,
                                     name=f"dT{i}") for i in range(KT)]
                if not f8:
                    wk_sb = [sb.tile([P, C], F16, tag="wt", bufs=40,
                                     name=f"wk{i}") for i in range(KT)]
                xT_sb = [sb.tile([P, SH], F16, tag="xa", bufs=8, name=f"xT{i}")
                         for i in range(KT)]
                # ---- SP load queue: strict phase-1 consumption order, then
                # next-phase prefetch.  Everything is resident well before use.
                if f8 == "kv":
                    for t in range(DT8):
                        nc.sync.dma_start(out=dT8_sb[t][:, :, 0:P],
                                          in_=dT8[t * P:(t + 1) * P, :, 0:P])
                        nc.sync.dma_start(out=wk8_sb[t][:], in_=wk8[t * P:(t + 1) * P, :, :])
                    for t in range(DT8):
                        nc.sync.dma_start(out=dT8_sb[t][:, :, P:SH],
                                          in_=dT8[t * P:(t + 1) * P, :, P:SH])
                    for t in range(DT8):
                        nc.sync.dma_start(out=wv8_sb[t][:], in_=wv8[t * P:(t + 1) * P, :, :])
                elif f8 == "k":
                    for t in range(DT8):
                        nc.sync.dma_start(out=dT8_sb[t][:, :, 0:P],
                                          in_=dT8[t * P:(t + 1) * P, :, 0:P])
                        nc.sync.dma_start(out=wk8_sb[t][:], in_=wk8[t * P:(t + 1) * P, :, :])
                    for i in range(KT):
                        nc.sync.dma_start(out=dT_sb[i][:, 0:P], in_=dT[i * P:(i + 1) * P, 0:P])
                    for t in range(DT8):
                        nc.sync.dma_start(out=dT8_sb[t][:, :, P:SH],
                                          in_=dT8[t * P:(t + 1) * P, :, P:SH])
                    for i in range(KT):
                        nc.sync.dma_start(out=dT_sb[i][:, P:SH], in_=dT[i * P:(i + 1) * P, P:SH])
                    for i in range(KT):
                        nc.sync.dma_start(out=wv_sb[i][:], in_=wv[i * P:(i + 1) * P, :])
                else:
                    for i in range(KT):
                        nc.sync.dma_start(out=dT_sb[i][:, 0:P], in_=dT[i * P:(i + 1) * P, 0:P])
                        nc.sync.dma_start(out=wk_sb[i][:], in_=wk[i * P:(i + 1) * P, :])
                    for i in range(KT):
                        nc.sync.dma_start(out=dT_sb[i][:, P:SH], in_=dT[i * P:(i + 1) * P, P:SH])
                    for i in range(KT):
                        nc.sync.dma_start(out=wv_sb[i][:], in_=wv[i * P:(i + 1) * P, :])
                bq_sb = sb.tile([P, MT], F32, tag="bias", bufs=3, name="bq_sb")
                b1_sb = sb.tile([P, MT], F32, tag="bias", bufs=3, name="b1_sb")
                b2_sb = sb.tile([P, MT], F32, tag="bias", bufs=3, name="b2_sb")
                nc.sync.dma_start(out=bq_sb[:], in_=bqt[:])
                nc.sync.dma_start(out=b1_sb[:], in_=b1t[:])
                nc.sync.dma_start(out=b2_sb[:], in_=b2t[:])
                if with_bkv:
                    bk_sb = sb.tile([1, C], F16, tag="brow", bufs=2, name="bk_sb")
                    bv_sb = sb.tile([1, C], F16, tag="brow", bufs=2, name="bv_sb")
                    ones = sb.tile([1, P], F16, tag="ones", bufs=1, name="ones")
                    nc.sync.dma_start(out=bk_sb[:], in_=bkr[:])
                    nc.sync.dma_start(out=bv_sb[:], in_=bvr[:])
                    nc.vector.memset(ones[:], 1.0)
                wq_sb = [sb.tile([P, C], F16, tag="wt", bufs=40, name=f"wq{i}")
                         for i in range(KT)]
                w1_sb = [sb.tile([P, C], F16, tag="wt", bufs=40, name=f"w1{i}")
                         for i in range(KT)]
                w2_sb = [sb.tile([P, C], F16, tag="wt", bufs=40, name=f"w2{i}")
                         for i in range(KT)]
                for i in range(KT):
                    nc.sync.dma_start(out=xT_sb[i][:], in_=xT[i * P:(i + 1) * P, :])
                    nc.sync.dma_start(out=wq_sb[i][:], in_=wq[i * P:(i + 1) * P, :])
                for i in range(KT):
                    nc.sync.dma_start(out=w1_sb[i][:], in_=w1[i * P:(i + 1) * P, :])
                for i in range(KT):
                    nc.sync.dma_start(out=w2_sb[i][:], in_=w2[i * P:(i + 1) * P, :])

                # ---- phase 1: K/V (sequence-major) and kv partial ----
                # Every stationary serves both feature halves (paired matmuls
                # into two PSUM banks) so the PE loads each stationary once
                # per two matmuls.  kv pairs two heads per [128,128] k
                # stationary; the cross-head half of each product lands in
                # the unused partition half of its bank (bank A even heads at
                # partitions 0:64, bank B odd heads at 64:128, pair i at
                # free i*128).  kv matmuls are interleaved into the next
                # projection's stream via `extras` so their LDWEIGHTS hide.
                kv_ps_a = psp.tile([P, NB], F32, tag="acc", bufs=2, name="kv_ps_a")
                kv_ps_b = psp.tile([P, NB], F32, tag="acc", bufs=2, name="kv_ps_b")

                def proj_seq(w_sb, brow, st, nm, extras=None):
                    ssl = slice(st * P, (st + 1) * P)
                    t = sb.tile([P, C], F16, tag="kvs", bufs=5, name=nm)
                    pk0 = psp.tile([P, NB], F32, tag="mm", bufs=6, name="pk0")
                    pk1 = psp.tile([P, NB], F32, tag="mm", bufs=6, name="pk1")
                    ex = list(extras) if extras else []
                    for kt in range(KT):
                        for nb, pk in ((0, pk0), (1, pk1)):
                            nc.tensor.matmul(
                                pk[:],
                                lhsT=dT_sb[kt][:, ssl],
                                rhs=w_sb[kt][:, nb * NB:(nb + 1) * NB],
                                start=(kt == 0),
                                stop=(kt == KT - 1 and not with_bkv),
                            )
                            if ex:
                                ex.pop(0)()
                    if with_bkv:
                        nc.tensor.matmul(pk0[:], lhsT=ones[:],
                                         rhs=brow[:, 0:NB],
                                         start=False, stop=True)
                        nc.tensor.matmul(pk1[:], lhsT=ones[:],
                                         rhs=brow[:, NB:C],
                                         start=False, stop=True)
                    nc.scalar.copy(t[:, 0:NB], pk0[:])
                    nc.vector.tensor_copy(t[:, NB:C], pk1[:])
                    return t

                def proj_seq_f8(w8_sb_, st, nm, extras=None):
                    # DoubleRow: contraction 256 per pass, stationary is the
                    # fp8 data slice [128,2,128], moving the fp8 weight
                    # [128,2,512].  Evictions undo the 2^15 pre-scale.
                    t = sb.tile([P, C], F16, tag="kvs", bufs=5, name=nm)
                    pk0 = psp.tile([P, NB], F32, tag="mm", bufs=6, name="pk0")
                    pk1 = psp.tile([P, NB], F32, tag="mm", bufs=6, name="pk1")
                    ex = list(extras) if extras else []
                    for tt in range(DT8):
                        for nb, pk in ((0, pk0), (1, pk1)):
                            nc.tensor.matmul(
                                pk[:],
                                lhsT=dT8_sb[tt][:, :, st * P:(st + 1) * P],
                                rhs=w8_sb_[tt][:, :, nb * NB:(nb + 1) * NB],
                                start=(tt == 0),
                                stop=(tt == DT8 - 1),
                                perf_mode=DRM,
                            )
                            # one kv matmul per DR matmul: its 107ns
                            # LDWEIGHTS hides under the 241ns DR stream
                            if ex:
                                ex.pop(0)()
                    nc.scalar.activation(t[:, 0:NB], pk0[:], AF.Copy, scale=SINV)
                    nc.vector.tensor_scalar_mul(t[:, NB:C], pk1[:], SINV)
                    return t

                def kv_thunks(k_t, v_t, st):
                    # pair i: full [128,128] product of the pair's k and v
                    # slices.  Quadrants [0:64,0:64] and [64:128,64:128] are
                    # the two heads' kv; the off-diagonal quadrants are
                    # cross-head junk, zeroed after the AllReduce.
                    def mk(i):
                        def go():
                            tgt = kv_ps_a if i < 4 else kv_ps_b
                            fo = (i % 4) * 128
                            nc.tensor.matmul(
                                tgt[:, fo:fo + 128],
                                lhsT=k_t[:, i * 128:(i + 1) * 128],
                                rhs=v_t[:, i * 128:(i + 1) * 128],
                                start=(st == 0 and i % 4 == 0),
                                stop=(st == MT - 1 and i % 4 == 3),
                            )
                        return go
                    return [mk(i) for i in range(8)]

                def PROJ_K(st, extras=None):
                    if f8:
                        return proj_seq_f8(wk8_sb, st, "k_t", extras)
                    return proj_seq(wk_sb, bk_sb if with_bkv else None, st,
                                    "k_t", extras)

                def PROJ_V(st, extras=None):
                    if f8 == "kv":
                        return proj_seq_f8(wv8_sb, st, "v_t", extras)
                    return proj_seq(wv_sb, bv_sb if with_bkv else None, st,
                                    "v_t", extras)

                pend = None
                for st in range(MT):
                    k_t = PROJ_K(st)
                    v_t = PROJ_V(st, extras=kv_thunks(*pend) if pend else None)
                    pend = (k_t, v_t, st)
                kv7 = kv_thunks(*pend)

                # ---- phase 2: AllReduce kv across the pair ----
                # kv_sb [128, 1024]: pair i at cols i*128 (junk rides along;
                # junk+junk is still junk and gets zeroed after the reduce).
                kv_sb = sb.tile([P, C], F16, tag="kvio", bufs=2, name="kv_sb")
                kv_in = drp.tile([P, C], F16, tag="dri", bufs=1, name="kv_in")
                kv_out = drp.tile([P, C], F16, tag="dro", bufs=1, name="kv_out")
                kv_red = sb.tile([P, C], F16, tag="kvr", bufs=2, name="kv_red")

                def kv_exchange():
                    nc.vector.tensor_copy(kv_sb[:, 0:NB], kv_ps_a[:])
                    nc.vector.tensor_copy(kv_sb[:, NB:C], kv_ps_b[:])
                    nc.gpsimd.dma_start(out=kv_in[:], in_=kv_sb[:])
                    if n_cores == 1 or not use_cc:
                        # single-core analysis build: stand in for the AllReduce
                        nc.gpsimd.dma_start(out=kv_out[:], in_=kv_in[:])
                    else:
                        nc.gpsimd.collective_compute(
                            "AllReduce", ALU.add, replica_groups=groups,
                            ins=[kv_in.opt()], outs=[kv_out.opt()],
                        )
                    nc.gpsimd.dma_start(out=kv_red[:], in_=kv_out[:])
                    # zero the cross-head quadrants so attention can contract
                    # over the full 128 partitions in one matmul per pair
                    for i in range(8):
                        co = i * 128
                        nc.vector.memset(kv_red[64:128, co:co + 64], 0.0)
                        nc.vector.memset(kv_red[0:64, co + 64:co + 128], 0.0)

                if phases <= 1:
                    for th in kv7:
                        th()
                    kv_exchange()
                    stg0 = sb.tile([P, NB], ODT, tag="stg", bufs=3, name="stg0")
                    nc.vector.tensor_copy(stg0[:], kv_red[:, 0:NB])
                    for m in range(MT):
                        for nb in range(NBLK):
                            nsl = slice(nb * NB, (nb + 1) * NB)
                            nc.gpsimd.dma_start(out=outT[m * P:(m + 1) * P, nsl],
                                                in_=stg0[:])
                    return

                # ---- phase 3: Q projection (all blocks), then attention ----
                # All 16 Q blocks precede the attention matmuls in PE order,
                # so the AllReduce has the full ~27us Q window to complete.
                # The last sequence block's kv matmuls ride in the first Q
                # blocks (from the 3rd stationary on, so the v_t eviction has
                # time to land).
                attn_sb = [sb.tile([P, SH], F16, tag="at", bufs=8, name=f"attn{m}")
                           for m in range(MT)]
                exchanged = []

                def q_extra():
                    if kv7:
                        kv7.pop(0)()
                    elif not exchanged:
                        exchanged.append(1)
                        kv_exchange()

                qpend = []
                nstat = 0
                for m in range(MT):
                    qp0 = psp.tile([P, NB], F32, tag="mm", bufs=6, name="qp0")
                    qp1 = psp.tile([P, NB], F32, tag="mm", bufs=6, name="qp1")
                    for kt in range(KT):
                        for nb, qp in ((0, qp0), (1, qp1)):
                            nc.tensor.matmul(
                                qp[:],
                                lhsT=wq_sb[kt][:, m * P:(m + 1) * P],
                                rhs=xT_sb[kt][:, nb * NB:(nb + 1) * NB],
                                start=(kt == 0), stop=(kt == KT - 1),
                            )
                        nstat += 1
                        if nstat >= 3:
                            q_extra()
                    for nb, qp in ((0, qp0), (1, qp1)):
                        qt = sb.tile([P, NB], F16, tag="qt", bufs=17, name="qt")
                        nc.scalar.activation(qt[:], qp[:], AF.Identity,
                                             bias=bq_sb[:, m:m + 1], scale=1.0)
                        qpend.append((m, nb, qt))
                while kv7:
                    kv7.pop(0)()
                if not exchanged:
                    kv_exchange()
                # nb-major attention order: the 8 nb=0 evictions complete
                # first, so MLP1's nb=0 sweep (which contracts over all 8
                # pairs at one sequence block) starts ~5us earlier.
                qpend.sort(key=lambda t: (t[1], t[0]))
                for m, nb, qt in qpend:
                    # qt partitions are [head 2m d | head 2m+1 d], matching
                    # the kv pair-block rows; with the junk quadrants zeroed
                    # one 128-contraction matmul yields both heads' attn in
                    # the right partition layout.
                    nsl = slice(nb * NB, (nb + 1) * NB)
                    ap_ = psp.tile([P, NB], F32, tag="mm", bufs=6, name="ap_")
                    nc.tensor.matmul(
                        ap_[:],
                        lhsT=kv_red[:, m * 128:(m + 1) * 128],
                        rhs=qt[:], start=True, stop=True)
                    # alternate eviction engines so the 8 evictions MLP1's
                    # first sweep waits on land in ~1.7us instead of 3.4us
                    if m % 2:
                        nc.scalar.copy(attn_sb[m][:, nsl], ap_[:])
                    else:
                        nc.vector.tensor_copy(attn_sb[m][:, nsl], ap_[:])

                if phases <= 2:
                    for m in range(MT):
                        for nb in range(NBLK):
                            nsl = slice(nb * NB, (nb + 1) * NB)
                            nc.gpsimd.dma_start(out=outT[m * P:(m + 1) * P, nsl],
                                                in_=attn_sb[m][:, nsl])
                    return

                # ---- phase 5: MLP hidden with fused exact GELU ----
                h1_sb = [sb.tile([P, SH], F16, tag="h1", bufs=8, name=f"h1{m}")
                         for m in range(MT)]
                for nb in range(NBLK):
                    nsl = slice(nb * NB, (nb + 1) * NB)
                    for m in range(MT):
                        hp = psp.tile([P, NB], F32, tag="mm", bufs=6, name="hp")
                        for kt in range(KT):
                            nc.tensor.matmul(
                                hp[:],
                                lhsT=w1_sb[kt][:, m * P:(m + 1) * P],
                                rhs=attn_sb[kt][:, nsl],
                                start=(kt == 0), stop=(kt == KT - 1),
                            )
                        nc.scalar.activation(h1_sb[m][:, nsl], hp[:],
                                             AF.Gelu if gelu else AF.Identity,
                                             bias=b1_sb[:, m:m + 1], scale=1.0)

                if phases <= 3:
                    for m in range(MT):
                        for nb in range(NBLK):
                            nsl = slice(nb * NB, (nb + 1) * NB)
                            nc.gpsimd.dma_start(out=outT[m * P:(m + 1) * P, nsl],
                                                in_=h1_sb[m][:, nsl])
                    return

                # fold the attention output into the residual so the MLP2
                # eviction is a single DVE op per block
                for m in range(MT):
                    nc.vector.tensor_add(xT_sb[m][:], xT_sb[m][:],
                                         attn_sb[m][:])

                # ---- phase 6: MLP out + residuals; stores on Pool queue ----
                for nb in range(NBLK):
                    nsl = slice(nb * NB, (nb + 1) * NB)
                    for m in range(MT):
                        op = psp.tile([P, NB], F32, tag="mm", bufs=6, name="op")
                        for kt in range(KT):
                            nc.tensor.matmul(
                                op[:],
                                lhsT=w2_sb[kt][:, m * P:(m + 1) * P],
                                rhs=h1_sb[kt][:, nsl],
                                start=(kt == 0), stop=(kt == KT - 1),
                            )
                        stg = sb.tile([P, NB], ODT, tag="stg", bufs=3, name="stg")
                        # stg = (op + b2) + (x + attn)   [attn pre-folded into xT]
                        nc.vector.scalar_tensor_tensor(
                            stg[:], op[:], b2_sb[:, m:m + 1],
                            xT_sb[m][:, nsl], op0=ALU.add, op1=ALU.add)
                        nc.gpsimd.dma_start(
                            out=outT[m * P:(m + 1) * P, nsl], in_=stg[:])

            # Straight-line unroll for timing runs (collectives cannot sit
            # inside a hardware For_i loop on this execution path).
            for _ in range(loop_r):
                body()

    nc.compile()
    return nc


def _get_program(with_bkv: bool, loop_r: int = 1, use_cc: bool = True,
                 phases: int = 4, f8: str = F8):
    key = (with_bkv, loop_r, use_cc, phases, f8)
    if key not in _CACHE:
        _CACHE[key] = _build(with_bkv, loop_r, use_cc=use_cc, phases=phases,
                             f8=f8)
    return _CACHE[key]


def _pack3(a8):
    """[C, w] (contraction-major) -> [DT8*128, 2, w] DoubleRow interleave."""
    cw = a8.shape[1]
    return np.ascontiguousarray(
        a8.reshape(DT8, 2, P, cw).transpose(0, 2, 1, 3).reshape(DT8 * P, 2, cw)
    )


def _pack_inputs(x, data, Wq, bq, Wk, bk, Wv, bv, W1, b1, W2, b2, with_bkv,
                 f8: str = F8):
    import ml_dtypes
    f32 = np.float32
    f16 = {"f16": np.float16, "bf16": ml_dtypes.bfloat16,
           "f32r": np.float32}[GDT]
    f8dt = ml_dtypes.float8_e4m3
    wq_s = np.ascontiguousarray(np.asarray(Wq, f32) * f32(SCALE), dtype=f32).astype(f16)
    w1_c = np.asarray(W1, f32).astype(f16)
    w2_c = np.asarray(W2, f32).astype(f16)
    if f8:
        wkf = np.asarray(Wk, f32) * f32(SW8)
        assert np.abs(wkf).max() < 240.0, "Wk fp8 overflow"
        wk8_c = _pack3(wkf.astype(f8dt))
    else:
        wk_c = np.asarray(Wk, f32).astype(f16)
    if f8 == "kv":
        wvf = np.asarray(Wv, f32) * f32(SW8)
        assert np.abs(wvf).max() < 240.0, "Wv fp8 overflow"
        wv8_c = _pack3(wvf.astype(f8dt))
    else:
        wv_c = np.asarray(Wv, f32).astype(f16)
    bqt = np.ascontiguousarray((np.asarray(bq, f32) * f32(SCALE)).reshape(MT, P).T)
    b1t = np.ascontiguousarray(np.asarray(b1, f32).reshape(MT, P).T)
    b2t = np.ascontiguousarray(np.asarray(b2, f32).reshape(MT, P).T)
    in_maps = []
    for c in range(8):
        b_, j = divmod(c, 2)
        dTf = np.ascontiguousarray(np.asarray(data, f32)[b_, j * SH:(j + 1) * SH, :].T)
        m = {
            "xT": np.ascontiguousarray(np.asarray(x, f32)[b_, j * SH:(j + 1) * SH, :].T).astype(f16),
            "wq": wq_s, "w1": w1_c, "w2": w2_c,
            "bqt": bqt, "b1t": b1t, "b2t": b2t,
        }
        if f8:
            d8 = dTf * f32(SD8)
            assert np.abs(d8).max() < 240.0, "data fp8 overflow"
            m["dT8"] = _pack3(d8.astype(f8dt))
            m["wk8"] = wk8_c
        else:
            m["wk"] = wk_c
        if f8 == "kv":
            m["wv8"] = wv8_c
        else:
            m["wv"] = wv_c
        if f8 != "kv":
            m["dT"] = dTf.astype(f16)
        if with_bkv:
            m["bkr"] = np.asarray(bk, f32).reshape(1, C).astype(f16)
            m["bvr"] = np.asarray(bv, f32).reshape(1, C).astype(f16)
        in_maps.append(m)
    return in_maps


def run_on_hw(inputs, loop_r: int = 1, trace: bool = False):
    """Run the SPMD program; returns BassKernelResults."""
    from concourse.bass_utils import run_bass_kernel_spmd

    with_bkv = bool(
        np.any(np.asarray(inputs["bk"])) or np.any(np.asarray(inputs["bv"]))
    )
    f8 = "" if with_bkv else F8
    nc = _get_program(with_bkv, loop_r, f8=f8)
    in_maps = _pack_inputs(
        inputs["x"], inputs["data"], inputs["Wq"], inputs["bq"], inputs["Wk"],
        inputs["bk"], inputs["Wv"], inputs["bv"], inputs["W1"], inputs["b1"],
        inputs["W2"], inputs["b2"], with_bkv, f8=f8,
    )
    res = run_bass_kernel_spmd(nc, in_maps, list(range(8)), trace=trace)
    return res


def kernel(**inputs) -> np.ndarray:
    res = run_on_hw(inputs, loop_r=1)
    out = np.empty((B, S, C), dtype=np.float32)
    for c in range(8):
        b_, j = divmod(c, 2)
        out[b_, j * SH:(j + 1) * SH, :] = res.results[c]["outT"].T.astype(np.float32)
    return out
